# revision 57
# baseline (speedup 1.0000x reference)
"""Curvphormer GNN layer as a Bass/Tile SPMD kernel for TRN2 (V6).

Design (per core c of NCORES, owning 49 windows x 128 nodes):
 - Edges sharded by src range. Pass 1 groups edges by 256-node tgt
   wide-windows (ww), sorted by src within a ww so pass-2 runs are
   contiguous. Pass 2 groups edges by own src-window as runs of R=4
   consecutive pass-1 slots (one indirect DMA per run-group reads ex).
 - Phase A: batched LN stats; per-window xn^T via PE transpose; q/k/v as
   bf16 matmuls to DRAM. AllGather(k) and AllGather(v) (both with Shared
   pair-HBM outputs); the k "hi" table is an offset view k_full[NCUT:]
   so int16 gather indices stay in range (no copy).
 - Pass 1 (super-blocks of 8 wws): q/k rows via batched NON-transposed
   dma_gather (slot-major, contiguous 256B writes — much faster on HW
   than transposed gathers); prod = q*k on DVE; per-head dot via strided
   DVE tensor_reduce; score = qk + PSUM(curv@4Wc + 4bc) added on DVE;
   batched exp(0.25*s). Denominator via half-one-hot rows (i128z, 256B)
   gathered once per super-block + lo/hi-masked ex matmuls into PSUM.
 - AllReduce(den, fp32, Shared out); full gathered v normalized LOCALLY
   on DVE (no second AllGather exposed between pass 1 and pass 2).
 - Pass 2 (32-tile blocks): vn gathered in node-pairs (idx=tgt//2, 512B
   elems) with parity folded into ex masks; ex via indirect run-gathers;
   aggregation via iota/is_equal one-hot matmuls into per-window PSUM;
   out = x + agg@Wo + FFN(LN2(x1)) with transposed-chunk FFN.
"""

import sys
if "/opt/trn_rl_repo" not in sys.path:
    sys.path.insert(0, "/opt/trn_rl_repo")

import numpy as np

import concourse.bass as bass
import concourse.mybir as mybir
from concourse.masks import make_identity

F32 = mybir.dt.float32
BF16 = mybir.dt.bfloat16
I32 = mybir.dt.int32
I16 = mybir.dt.int16

D = 128
H = 8
HD = 16
LN_EPS = 1e-5
R2 = 4          # pass-2 run length (ex rows per indirect read)
NCUT = 17408    # k_hi table base (68 wide-windows * 256)


def _bf(a):
    import ml_dtypes
    return np.asarray(a, np.float32).astype(ml_dtypes.bfloat16)


def _wrap16(flat, ncols):
    """int16 idx table [128, ncols]: slot s -> [s%16, s//16], replicated."""
    tab = np.zeros((128, ncols), np.int16)
    tab[:16] = np.asarray(flat, np.int16).reshape(-1, 16).T
    for r in range(16, 128, 16):
        tab[r:r + 16] = tab[:16]
    return tab


class P:
    def __init__(self, ncores, W, T1W, NG2W):
        self.ncores = ncores
        self.W = W
        self.T1W = T1W
        self.NWW = ncores * W // 2
        self.T1 = self.NWW * T1W
        self.NG2W = NG2W
        self.T2W = NG2W * R2
        self.T2 = W * self.T2W
        self.NG2 = W * NG2W
        self.nodes_pc = W * 128
        self.npad = ncores * W * 128


# --------------------------------------------------------------------------
# Host-side preprocessing
# --------------------------------------------------------------------------

def host_prep(x, edge_index, curv, weights, ncores, W):
    N = x.shape[0]
    E = edge_index.shape[1]
    nodes_pc = W * 128
    npad = ncores * nodes_pc
    assert npad >= N

    src = np.asarray(edge_index[0], dtype=np.int64)
    tgt = np.asarray(edge_index[1], dtype=np.int64)
    x_pad = np.zeros((npad, D), dtype=np.float32)
    x_pad[:N] = x

    core_of = (src // 128) // W
    order_by_core = np.argsort(core_of, kind="stable")
    counts = np.bincount(core_of, minlength=ncores)
    splits = np.split(order_by_core, np.cumsum(counts)[:-1])

    NWW = ncores * W // 2

    # ---- pass-1 slot assignment (per core), sorted by (ww, src) ----
    T1W = 0
    p1_orders = []
    for c in range(ncores):
        e_c = splits[c]
        ww_of = tgt[e_c] // 256
        order = np.lexsort((src[e_c], ww_of))
        e_sorted = e_c[order]
        p1_orders.append(e_sorted)
        cnt = np.bincount(tgt[e_sorted] // 256, minlength=NWW)
        T1W = max(T1W, int(np.ceil(cnt.max() / 128)))
    T1 = NWW * T1W
    S1 = T1 * 128

    # per-core pass-1 tables
    core_p1 = []
    for c in range(ncores):
        e_sorted = p1_orders[c]
        ww_sorted = tgt[e_sorted] // 256
        cnt = np.bincount(ww_sorted, minlength=NWW)
        starts = NWW and np.concatenate([[0], np.cumsum(cnt)[:-1]])
        slots = np.zeros(len(e_sorted), np.int64)
        for ww in range(NWW):
            k = cnt[ww]
            if k:
                sl = ww * T1W * 128 + np.arange(k)
                slots[starts[ww]:starts[ww] + k] = sl
        real1 = np.zeros(S1, bool)
        real1[slots] = True
        tgt1 = np.zeros(S1, np.int64)
        tgt1[slots] = tgt[e_sorted]
        src1 = np.zeros(S1, np.int64)
        src1[slots] = src[e_sorted]
        core_p1.append((e_sorted, slots, real1, tgt1, src1))

    # ---- pass-2 runs (per core, per window), split lo/hi by tgt ww ----
    NGL = NGH = 0
    core_runs = []
    for c in range(ncores):
        e_sorted, slots, real1, tgt1, src1 = core_p1[c]
        w_loc = (src[e_sorted] // 128) - c * W
        runs_per_w = []
        for w in range(W):
            s_list = np.sort(slots[w_loc == w])
            if len(s_list) == 0:
                runs_per_w.append(([], []))
                continue
            d = np.diff(s_list)
            segstart = np.concatenate([[0], np.flatnonzero(d != 1) + 1])
            seglen = np.diff(np.concatenate([segstart, [len(s_list)]]))
            lo_runs, hi_runs = [], []
            for ss, ln in zip(segstart, seglen):
                s0r = int(s_list[ss])
                is_hi = tgt1[s0r] >= NCUT  # run is ww-pure
                for off in range(0, ln, R2):
                    r = (int(s_list[ss + off]), int(min(R2, ln - off)))
                    (hi_runs if is_hi else lo_runs).append(r)
            runs_per_w.append((lo_runs, hi_runs))
            NGL = max(NGL, (len(lo_runs) + 127) // 128)
            NGH = max(NGH, (len(hi_runs) + 127) // 128)
        core_runs.append(runs_per_w)
    NG2W = NGL + NGH

    pp = P(ncores, W, T1W, NG2W)
    pp.NGL = NGL
    T2, T2W, NG2 = pp.T2, pp.T2W, pp.NG2
    S2 = T2 * 128

    # ---- weights (common) ----
    g1, be1 = weights["g1"], weights["be1"]
    g2, be2 = weights["g2"], weights["be2"]

    def foldA(Wm, b):
        return (_bf(g1[:, None] * Wm),
                _bf((be1 @ Wm + b))[None, :])

    wqp, bqp = foldA(weights["Wq"], weights["bq"])
    wkp, bkp = foldA(weights["Wk"], weights["bk"])
    wvp, bvp = foldA(weights["Wv"], weights["bv"])
    w1g = _bf(g2[:, None] * weights["W1"])
    b12 = (be2 @ weights["W1"] + weights["b1"]).astype(np.float32)
    b12cols = np.ascontiguousarray(b12.reshape(4, 128).T)  # [128, 4]
    w2ch = _bf(np.ascontiguousarray(
        weights["W2"].astype(np.float32).reshape(4, 128, D)
        .transpose(1, 0, 2).reshape(128, 4 * D)))

    headmask = np.zeros((128, H), np.float32)
    for f in range(128):
        headmask[f, f // 16] = 1.0
    i128z = np.zeros((129, 128), np.float32)
    i128z[:128, :128] = np.eye(128)
    iota128 = np.tile(np.arange(128, dtype=np.float32)[None, :], (128, 1))

    common = {
        "wqp": wqp, "wkp": wkp, "wvp": wvp,
        "bqp": bqp, "bkp": bkp, "bvp": bvp,
        "wc4": _bf(4.0 * weights["Wc"]),
        "bc4": _bf(4.0 * weights["bc"])[None, :],
        "wo_b": _bf(weights["Wo"]), "bo_b": _bf(weights["bo"])[None, :],
        "w1g": w1g, "b12cols": b12cols.astype(np.float32),
        "w2ch": w2ch, "b2_b": _bf(weights["b2"])[None, :],
        "ones_b": np.ones((1, D), np.float32).astype(np.float32),
        "headmask": _bf(headmask),
        "i128z": _bf(i128z),
        "iota128_b": _bf(iota128),
    }
    common["ones_b"] = _bf(np.ones((1, D), np.float32))

    in_maps = []
    for c in range(ncores):
        e_sorted, slots, real1, tgt1, src1 = core_p1[c]

        qi = np.where(real1, src1 - c * nodes_pc, 0)
        klo = np.where(tgt1 < NCUT, tgt1, tgt1 - NCUT)
        ki = np.where(real1, klo, 0)
        ohi = np.where(real1, tgt1 % 128, 128)
        is_lo = real1 & ((tgt1 % 256) < 128)
        is_hi = real1 & ((tgt1 % 256) >= 128)
        lobm = np.ascontiguousarray(
            is_lo.astype(np.float32).reshape(T1, 128).T)
        hibm = np.ascontiguousarray(
            is_hi.astype(np.float32).reshape(T1, 128).T)

        curv1 = np.zeros((S1, D), np.float32)
        curv1[slots] = curv[e_sorted]
        c1t = curv1.reshape(T1, 128, D).transpose(0, 2, 1)  # [T1, D, 128]
        curv1t = _bf(np.ascontiguousarray(
            c1t.reshape(T1 // 2, 2, D, 128).transpose(0, 2, 1, 3))
            .reshape((T1 // 2) * D, 256))

        # ---- pass 2 tables ----
        runs_per_w = core_runs[c]
        inv_slot1 = np.full(S1, -1, np.int64)
        inv_slot1[slots] = e_sorted
        vni = np.zeros(S2, np.int64)
        sl2 = np.full(S2, -1.0, np.float32)
        valid2 = np.zeros(S2, bool)
        exoff = np.full((128, NG2), T1 * 128, np.int32)
        for w in range(W):
            lo_runs, hi_runs = runs_per_w[w]
            for sec, g0 in ((lo_runs, 0), (hi_runs, NGL)):
                for ri, (r0, ln) in enumerate(sec):
                    g, p = g0 + ri // 128, ri % 128
                    exoff[p, w * NG2W + g] = r0
                    base_tile = (w * NG2W + g) * R2
                    for t_ in range(ln):
                        e = inv_slot1[r0 + t_]
                        u = (base_tile + t_) * 128 + p
                        vni[u] = tgt[e] if g0 == 0 else tgt[e] - NCUT
                        sl2[u] = float(src[e] - (c * W + w) * 128)
                        valid2[u] = True

        x_own = np.ascontiguousarray(x_pad[c * nodes_pc:(c + 1) * nodes_pc])

        m = dict(common)
        m.update({
            "x_own": x_own,
            "x_bf": _bf(x_own),
            "curv1t": curv1t,
            "lobm": _bf(lobm),
            "hibm": _bf(hibm),
            "qi16": _wrap16(qi, T1 * 8),
            "ki16": _wrap16(ki, T1 * 8),
            "ohi16": _wrap16(ohi, T1 * 8),
            "vni16": _wrap16(vni, T2 * 8),
            "exoff": exoff,
            "srcl2": _bf(np.ascontiguousarray(
                sl2.reshape(T2, 128).T)),
            "blm": _bf(np.ascontiguousarray(
                valid2.astype(np.float32).reshape(T2, 128).T)),
        })
        in_maps.append(m)

    return pp, in_maps


# --------------------------------------------------------------------------
# Device program
# --------------------------------------------------------------------------

def declare_io(nc, pp):
    t = {}

    def din(name, shape, dt=F32):
        t[name] = nc.dram_tensor(name, list(shape), dt, kind="ExternalInput").ap()

    W, T1, T2 = pp.W, pp.T1, pp.T2
    din("x_own", (pp.nodes_pc, D))
    din("x_bf", (pp.nodes_pc, D), BF16)
    din("curv1t", ((T1 // 2) * 128, 2 * D), BF16)
    din("lobm", (128, T1), BF16)
    din("hibm", (128, T1), BF16)
    din("qi16", (128, T1 * 8), I16)
    din("ki16", (128, T1 * 8), I16)
    din("ohi16", (128, T1 * 8), I16)
    din("vni16", (128, T2 * 8), I16)
    din("exoff", (128, pp.NG2), I32)
    din("srcl2", (128, T2), BF16)
    din("blm", (128, T2), BF16)
    for n, shp, dt in [
            ("wqp", (D, D), BF16), ("wkp", (D, D), BF16), ("wvp", (D, D), BF16),
            ("bqp", (1, D), BF16), ("bkp", (1, D), BF16), ("bvp", (1, D), BF16),
            ("wc4", (D, H), BF16), ("bc4", (1, H), BF16),
            ("wo_b", (D, D), BF16), ("bo_b", (1, D), BF16),
            ("w1g", (D, 4 * D), BF16), ("b12cols", (128, 4), F32),
            ("w2ch", (D, 4 * D), BF16), ("b2_b", (1, D), BF16),
            ("ones_b", (1, D), BF16), ("headmask", (D, H), BF16),
            ("i128z", (129, 128), BF16), ("iota128_b", (128, 128), BF16)]:
        din(n, shp, dt)
    t["out"] = nc.dram_tensor("out", [pp.nodes_pc, D], F32,
                              kind="ExternalOutput").ap()
    return t


def build(tc, t, pp):
    nc = tc.nc
    W, T1W, T1, T2W, T2, NWW = (pp.W, pp.T1W, pp.T1, pp.T2W, pp.T2,
                                pp.NWW)
    NW = pp.ncores * W
    rg = [list(range(pp.ncores))]
    from contextlib import ExitStack
    ctx = ExitStack()

    # internal DRAM
    q_own_d, _ = tc.tile([pp.nodes_pc, D], BF16, space="DRAM", name="q_own_d")
    k_own_d, _ = tc.tile([pp.nodes_pc, D], BF16, space="DRAM", name="k_own_d")
    v_own_d, _ = tc.tile([pp.nodes_pc, D], BF16, space="DRAM", name="v_own_d")
    shpool = ctx.enter_context(tc.tile_pool(name="shdram", space="DRAM",
                                            bufs=1))
    k_full = shpool.tile([pp.npad, D], BF16, name="k_full",
                         addr_space="Shared")
    v_full = shpool.tile([pp.npad, D], BF16, name="v_full",
                         addr_space="Shared")
    den_d, _ = tc.tile([NW * 128, H], F32, space="DRAM", name="den_d")
    den_all = shpool.tile([NW * 128, H], F32, name="den_all",
                          addr_space="Shared")
    vn_full, _ = tc.tile([pp.npad, D], BF16, space="DRAM", name="vn_full")
    ex_d2, _ = tc.tile([T1 * 128 + 8, H], BF16, space="DRAM", name="ex_d2")

    const = ctx.enter_context(tc.tile_pool(name="const", bufs=1))

    def load_const(name):
        ap = t[name]
        tl = const.tile(list(ap.shape), ap.dtype, name=f"c_{name}")
        nc.sync.dma_start(tl[:], ap[:])
        return tl

    wqp_s = load_const("wqp"); wkp_s = load_const("wkp"); wvp_s = load_const("wvp")
    bqp_s = load_const("bqp"); bkp_s = load_const("bkp"); bvp_s = load_const("bvp")
    wc4_s = load_const("wc4"); bc4_s = load_const("bc4")
    wo_s = load_const("wo_b"); bo_s = load_const("bo_b")
    w1g_s = load_const("w1g"); b12c_s = load_const("b12cols")
    w2_s = load_const("w2ch"); b2_s = load_const("b2_b")
    ones_s = load_const("ones_b"); hmask_s = load_const("headmask")
    iota128_s = load_const("iota128_b")

    ident = const.tile([128, 128], F32, name="ident")
    make_identity(nc, ident[:])
    ident_b = const.tile([128, 128], BF16, name="ident_b")
    nc.vector.tensor_copy(out=ident_b[:], in_=ident[:])
    eps_col = const.tile([128, 1], F32, name="eps_col")
    nc.vector.memset(eps_col[:], LN_EPS)
    zrow = const.tile([8, H], BF16, name="zrow")
    nc.vector.memset(zrow[:], 0.0)

    # residents
    den_tab = const.tile([128, NWW * 2 * H], F32, name="den_tab")
    x1_res = const.tile([128, W * 128], F32, name="x1_res")

    # ---------------- Phase A ----------------
    with tc.tile_pool(name="pA", bufs=1) as pA, \
         tc.tile_pool(name="pAw", bufs=2) as pAw, \
         tc.tile_pool(name="pAp", bufs=2, space="PSUM") as pAp:
        xb = pA.tile([128, W * 128], BF16, tag="xb")
        nc.sync.dma_start(
            xb[:].rearrange("p (w f) -> p w f", w=W),
            t["x_bf"][:].rearrange("(w p) f -> p w f", p=128))
        xv = xb[:].rearrange("p (w f) -> p w f", w=W)
        s1 = pA.tile([128, W], F32, tag="s1")
        nc.vector.tensor_reduce(out=s1[:], in_=xv, axis=mybir.AxisListType.X,
                                op=mybir.AluOpType.add)
        sq = pA.tile([128, W * 128], BF16, tag="sq")
        nc.scalar.activation(out=sq[:], in_=xb[:],
                             func=mybir.ActivationFunctionType.Square)
        s2 = pA.tile([128, W], F32, tag="s2")
        nc.vector.tensor_reduce(out=s2[:],
                                in_=sq[:].rearrange("p (w f) -> p w f", w=W),
                                axis=mybir.AxisListType.X,
                                op=mybir.AluOpType.add)
        mcol = pA.tile([128, W], F32, tag="mcol")
        nc.vector.tensor_scalar_mul(mcol[:], s1[:], 1.0 / 128.0)
        m2c = pA.tile([128, W], F32, tag="m2c")
        nc.vector.tensor_tensor(out=m2c[:], in0=mcol[:], in1=mcol[:],
                                op=mybir.AluOpType.mult)
        var = pA.tile([128, W], F32, tag="var")
        nc.vector.scalar_tensor_tensor(out=var[:], in0=s2[:],
                                       scalar=1.0 / 128.0, in1=m2c[:],
                                       op0=mybir.AluOpType.mult,
                                       op1=mybir.AluOpType.subtract)
        stdc = pA.tile([128, W], F32, tag="stdc")
        nc.scalar.activation(out=stdc[:], in_=var[:],
                             func=mybir.ActivationFunctionType.Sqrt,
                             bias=eps_col[:])
        rstd = pA.tile([128, W], F32, tag="rstd")
        nc.vector.reciprocal(out=rstd[:], in_=stdc[:])
        negm = pA.tile([128, W], F32, tag="negm")
        nc.vector.tensor_scalar_mul(negm[:], mcol[:], -1.0)
        xnt = pA.tile([128, W * 128], BF16, tag="xnt")
        nc.vector.tensor_tensor(
            out=xnt[:].rearrange("p (w f) -> p w f", w=W), in0=xv,
            in1=negm[:].rearrange("p w -> p w ()").broadcast_to([128, W, 128]),
            op=mybir.AluOpType.add)
        xn = pA.tile([128, W * 128], BF16, tag="xn")
        nc.vector.tensor_tensor(
            out=xn[:].rearrange("p (w f) -> p w f", w=W),
            in0=xnt[:].rearrange("p (w f) -> p w f", w=W),
            in1=rstd[:].rearrange("p w -> p w ()").broadcast_to([128, W, 128]),
            op=mybir.AluOpType.mult)

        for w in range(W):
            xnT_ps = pAp.tile([128, 128], BF16, tag="xnT_ps")
            nc.tensor.transpose(out=xnT_ps[:], in_=xn[:, w * 128:(w + 1) * 128],
                                identity=ident_b[:])
            xnT = pAw.tile([128, 128], BF16, tag="xnT")
            nc.vector.tensor_copy(out=xnT[:], in_=xnT_ps[:])
            for nm, wmat, brow, dst in (
                    ("q", wqp_s, bqp_s, q_own_d), ("k", wkp_s, bkp_s, k_own_d),
                    ("v", wvp_s, bvp_s, v_own_d)):
                ps = pAp.tile([128, 128], F32, tag="ps")
                nc.tensor.matmul(out=ps[:], lhsT=xnT[:], rhs=wmat[:],
                                 start=True, stop=False)
                nc.tensor.matmul(out=ps[:], lhsT=ones_s[:], rhs=brow[:],
                                 start=False, stop=True)
                ot = pAw.tile([128, 128], BF16, tag=f"o_{nm}")
                nc.scalar.activation(out=ot[:], in_=ps[:],
                                     func=mybir.ActivationFunctionType.Copy)
                nc.sync.dma_start(dst[w * 128:(w + 1) * 128, :], ot[:])

    nc.gpsimd.collective_compute(
        "AllGather", mybir.AluOpType.bypass, replica_groups=rg,
        ins=[k_own_d.opt()], outs=[k_full.opt()])
    nc.gpsimd.collective_compute(
        "AllGather", mybir.AluOpType.bypass, replica_groups=rg,
        ins=[v_own_d.opt()], outs=[v_full.opt()])

    # ---------------- Pass 1 ----------------
    NBLK = NWW // 2
    nt1 = 2 * T1W
    BB = 4                        # blocks per super-block
    NSB = (NBLK + BB - 1) // BB
    WWCUT = NCUT // 256           # first hi wide-window
    k_hi_v = k_full[NCUT:pp.npad, :]
    with tc.tile_pool(name="p1", bufs=2) as p1, \
         tc.tile_pool(name="p1p", bufs=2, space="PSUM") as p1p, \
         tc.tile_pool(name="p1d", bufs=2, space="PSUM") as p1d:
        nc.sync.dma_start(ex_d2[T1 * 128:T1 * 128 + 8, :], zrow[:])
        for sbi in range(NSB):
            b0 = sbi * BB
            nb = min(BB, NBLK - b0)
            nt = nb * nt1
            t0 = b0 * nt1
            s0 = t0 * 128
            ni = nt * 128
            cvb = p1.tile([128, BB * nt1 * 128], BF16, tag="cvb")
            nc.sync.dma_start(
                cvb[:, :ni].rearrange("p (b e) -> p b e", e=256),
                t["curv1t"][(t0 // 2) * 128:((t0 + nt) // 2) * 128, :]
                .rearrange("(b p) e -> p b e", p=128))
            qi_s = p1.tile([128, BB * nt1 * 8], I16, tag="qi_s")
            nc.sync.dma_start(qi_s[:, :nt * 8],
                              t["qi16"][:, s0 // 16:(s0 + ni) // 16])
            ki_s = p1.tile([128, BB * nt1 * 8], I16, tag="ki_s")
            nc.sync.dma_start(ki_s[:, :nt * 8],
                              t["ki16"][:, s0 // 16:(s0 + ni) // 16])
            ohi_s = p1.tile([128, BB * nt1 * 8], I16, tag="ohi_s")
            nc.sync.dma_start(ohi_s[:, :nt * 8],
                              t["ohi16"][:, s0 // 16:(s0 + ni) // 16])
            lob_s = p1.tile([128, BB * nt1], BF16, tag="lob_s")
            nc.sync.dma_start(lob_s[:, :nt], t["lobm"][:, t0:t0 + nt])
            hib_s = p1.tile([128, BB * nt1], BF16, tag="hib_s")
            nc.sync.dma_start(hib_s[:, :nt], t["hibm"][:, t0:t0 + nt])
            qT = p1.tile([128, BB * nt1 * 128], BF16, tag="qT")
            nc.gpsimd.dma_gather(
                out_ap=qT[:, :ni].rearrange("p (i e) -> p i e", i=nt),
                in_ap=q_own_d[:], idxs_ap=qi_s[:, :ni // 16],
                num_idxs=ni, num_idxs_reg=ni, elem_size=128,
                single_packet=False)
            kT = p1.tile([128, BB * nt1 * 128], BF16, tag="kT")
            ww_lo, ww_hi = 2 * b0, 2 * (b0 + nb)
            segs = []
            if ww_lo < WWCUT:
                segs.append((ww_lo, min(ww_hi, WWCUT), k_full[:]))
            if ww_hi > WWCUT:
                segs.append((max(ww_lo, WWCUT), ww_hi, k_hi_v))
            for (wa, wb, ktab_ap) in segs:
                ta = (wa - ww_lo) * T1W
                tb = (wb - ww_lo) * T1W
                na = (tb - ta) * 128
                nc.gpsimd.dma_gather(
                    out_ap=kT[:, ta * 128:tb * 128]
                    .rearrange("p (i e) -> p i e", i=tb - ta),
                    in_ap=ktab_ap,
                    idxs_ap=ki_s[:, ta * 8:tb * 8],
                    num_idxs=na, num_idxs_reg=na, elem_size=128,
                    single_packet=False)
            ohb = p1.tile([128, BB * nt1 * 128], BF16, tag="ohb")
            nc.gpsimd.dma_gather(
                out_ap=ohb[:, :ni].rearrange("p (i e) -> p i e", i=nt),
                in_ap=t["i128z"][:], idxs_ap=ohi_s[:, :ni // 16],
                num_idxs=ni, num_idxs_reg=ni, elem_size=128,
                single_packet=False)
            prodT = p1.tile([128, BB * nt1 * 128], BF16, tag="prodT")
            nc.vector.tensor_tensor(out=prodT[:, :ni], in0=qT[:, :ni],
                                    in1=kT[:, :ni], op=mybir.AluOpType.mult)
            qkred = p1.tile([128, BB * nt1 * 8], F32, tag="qkred")
            nc.vector.tensor_reduce(
                out=qkred[:, :nt * 8].rearrange("p (b h) -> p b h", h=H),
                in_=prodT[:, :ni].rearrange("p (b h x) -> p b h x",
                                            b=nt, h=H),
                axis=mybir.AxisListType.X, op=mybir.AluOpType.add)
            exb = p1.tile([128, BB * nt1 * 8], BF16, tag="exb")
            for bl in range(nb):
                sc_ps = p1p.tile([128, nt1 * 8], F32, tag="sc_ps")
                for j0 in range(nt1):
                    j = bl * nt1 + j0
                    scj = sc_ps[:, j0 * 8:(j0 + 1) * 8]
                    nc.tensor.matmul(out=scj,
                                     lhsT=cvb[:, j * 128:(j + 1) * 128],
                                     rhs=wc4_s[:], start=True, stop=False)
                    nc.tensor.matmul(out=scj, lhsT=ones_s[:], rhs=bc4_s[:],
                                     start=False, stop=True)
                s_sb = p1.tile([128, nt1 * 8], F32, tag="s_sb")
                nc.vector.tensor_tensor(
                    out=s_sb[:],
                    in0=qkred[:, bl * nt1 * 8:(bl + 1) * nt1 * 8],
                    in1=sc_ps[:], op=mybir.AluOpType.add)
                nc.scalar.activation(
                    out=exb[:, bl * nt1 * 8:(bl + 1) * nt1 * 8],
                    in_=s_sb[:], func=mybir.ActivationFunctionType.Exp,
                    scale=0.25)
            exl = p1.tile([128, BB * nt1 * 8], BF16, tag="exl")
            exh = p1.tile([128, BB * nt1 * 8], BF16, tag="exh")
            nc.vector.tensor_tensor(
                out=exl[:, :nt * 8].rearrange("p (b h) -> p b h", h=H),
                in0=exb[:, :nt * 8].rearrange("p (b h) -> p b h", h=H),
                in1=lob_s[:, :nt].rearrange("p b -> p b ()")
                .broadcast_to([128, nt, H]),
                op=mybir.AluOpType.mult)
            nc.vector.tensor_tensor(
                out=exh[:, :nt * 8].rearrange("p (b h) -> p b h", h=H),
                in0=exb[:, :nt * 8].rearrange("p (b h) -> p b h", h=H),
                in1=hib_s[:, :nt].rearrange("p b -> p b ()")
                .broadcast_to([128, nt, H]),
                op=mybir.AluOpType.mult)
            ohv = ohb[:, :ni].rearrange("p (i e) -> p i e", i=nt)
            for bl in range(nb):
                for i in range(2):
                    ww = 2 * (b0 + bl) + i
                    psd_lo = p1d.tile([128, H], F32, tag="psd_lo",
                                      name="psd_lo")
                    psd_hi = p1d.tile([128, H], F32, tag="psd_hi",
                                      name="psd_hi")
                    for tt in range(T1W):
                        jj = bl * nt1 + i * T1W + tt
                        nc.tensor.matmul(out=psd_lo[:], lhsT=ohv[:, jj, :],
                                         rhs=exl[:, jj * 8:(jj + 1) * 8],
                                         start=(tt == 0),
                                         stop=(tt == T1W - 1))
                        nc.tensor.matmul(out=psd_hi[:], lhsT=ohv[:, jj, :],
                                         rhs=exh[:, jj * 8:(jj + 1) * 8],
                                         start=(tt == 0),
                                         stop=(tt == T1W - 1))
                    nc.vector.tensor_copy(
                        out=den_tab[:, ww * 2 * H:ww * 2 * H + H],
                        in_=psd_lo[:])
                    nc.vector.tensor_copy(
                        out=den_tab[:, ww * 2 * H + H:(ww + 1) * 2 * H],
                        in_=psd_hi[:])
            nc.sync.dma_start(
                ex_d2[s0:s0 + ni, :].rearrange("(b p) h -> p b h", p=128),
                exb[:, :nt * 8].rearrange("p (b h) -> p b h", h=H))

        nc.sync.dma_start(
            den_d[:].rearrange("(w p) h -> p w h", p=128),
            den_tab[:].rearrange("p (w h) -> p w h", h=H))

    nc.gpsimd.collective_compute(
        "AllReduce", mybir.AluOpType.add, replica_groups=rg,
        ins=[den_d.opt()], outs=[den_all.opt()])

    # ---------------- Phase C: normalize full gathered v locally ---------
    with tc.tile_pool(name="pC", bufs=2) as pC:
        CHV = 8192
        nchv = (pp.npad + CHV - 1) // CHV
        for ch in range(nchv):
            r0 = ch * CHV
            nr = min(CHV, pp.npad - r0)
            na = nr // 128
            vb = pC.tile([128, (CHV // 128) * 128], BF16, tag="vb")
            nc.sync.dma_start(
                vb[:, :na * 128].rearrange("p (a f) -> p a f", a=na),
                v_full[r0:r0 + nr, :].rearrange("(a p) f -> p a f", p=128))
            db = pC.tile([128, (CHV // 128) * H], F32, tag="db")
            nc.sync.dma_start(
                db[:, :na * H].rearrange("p (a h) -> p a h", a=na),
                den_all[r0:r0 + nr, :].rearrange("(a p) h -> p a h", p=128))
            nc.vector.tensor_scalar_max(db[:, :na * H], db[:, :na * H],
                                        1e-30)
            rec = pC.tile([128, (CHV // 128) * H], F32, tag="rec")
            nc.vector.reciprocal(out=rec[:, :na * H], in_=db[:, :na * H])
            vnb = pC.tile([128, (CHV // 128) * 128], BF16, tag="vnb")
            nc.vector.tensor_tensor(
                out=vnb[:, :na * 128]
                .rearrange("p (a h x) -> p a h x", a=na, h=H),
                in0=vb[:, :na * 128]
                .rearrange("p (a h x) -> p a h x", a=na, h=H),
                in1=rec[:, :na * H].rearrange("p (a h) -> p a h ()", a=na)
                .broadcast_to([128, na, H, HD]),
                op=mybir.AluOpType.mult)
            nc.sync.dma_start(
                vn_full[r0:r0 + nr, :].rearrange("(a p) f -> p a f", p=128),
                vnb[:, :na * 128].rearrange("p (a f) -> p a f", a=na))

    # ---------------- Pass 2 ----------------
    B2 = 32
    NB2 = (T2 + B2 - 1) // B2
    vn_hi_v = vn_full[NCUT:pp.npad, :]
    NGLT = pp.NGL * R2          # lo tiles per window
    with tc.tile_pool(name="p2", bufs=2) as p2, \
         tc.tile_pool(name="p2c", bufs=1) as p2c, \
         tc.tile_pool(name="p2p", bufs=2, space="PSUM") as p2p, \
         tc.tile_pool(name="p2a", bufs=2, space="PSUM") as p2a, \
         tc.tile_pool(name="pD", bufs=2) as pD:
        vni_s = p2c.tile([128, T2 * 8], I16, name="vni_s")
        nc.sync.dma_start(vni_s[:], t["vni16"][:])
        exoff_s = p2c.tile([128, pp.NG2], I32, name="exoff_s")
        nc.sync.dma_start(exoff_s[:], t["exoff"][:])
        srcl2_s = p2c.tile([128, T2], BF16, name="srcl2_s")
        nc.sync.dma_start(srcl2_s[:], t["srcl2"][:])
        blm_s = p2c.tile([128, T2], BF16, name="blm_s")
        nc.sync.dma_start(blm_s[:], t["blm"][:])

        aggT_cur = [None]
        for bi in range(NB2):
            t0 = bi * B2
            nt = min(B2, T2 - t0)
            s0 = t0 * 128
            ni = nt * 128
            vgb = p2.tile([128, B2 * 128], BF16, tag="vgb")
            segs = []
            for k in range(nt):
                v_ = 0 if ((t0 + k) % T2W) < NGLT else 1
                if segs and segs[-1][2] == v_:
                    segs[-1][1] = k + 1
                else:
                    segs.append([k, k + 1, v_])
            for ka, kb, v_ in segs:
                nn_ = (kb - ka) * 128
                nc.gpsimd.dma_gather(
                    out_ap=vgb[:, ka * 128:kb * 128]
                    .rearrange("p (i e) -> p i e", i=kb - ka),
                    in_ap=(vn_full[:] if v_ == 0 else vn_hi_v),
                    idxs_ap=vni_s[:, (s0 + ka * 128) // 16:
                                  (s0 + kb * 128) // 16],
                    num_idxs=nn_, num_idxs_reg=nn_, elem_size=128,
                    single_packet=False)
            egb = p2.tile([128, B2 * 8], BF16, tag="egb")
            ng = (nt + R2 - 1) // R2
            for gi in range(ng):
                g = t0 // R2 + gi
                nc.gpsimd.indirect_dma_start(
                    out=egb[:, gi * R2 * 8:(gi + 1) * R2 * 8],
                    out_offset=None,
                    in_=ex_d2[:],
                    in_offset=bass.IndirectOffsetOnAxis(
                        ap=exoff_s[:, g:g + 1], axis=0))
            exbl = p2.tile([128, B2 * 8], BF16, tag="exbl")
            nc.vector.tensor_tensor(
                out=exbl[:, :nt * 8].rearrange("p (b h) -> p b h", b=nt),
                in0=egb[:, :nt * 8].rearrange("p (b h) -> p b h", b=nt),
                in1=blm_s[:, t0:t0 + nt].rearrange("p b -> p b ()")
                .broadcast_to([128, nt, H]),
                op=mybir.AluOpType.mult)
            msg = p2.tile([128, B2 * 128], BF16, tag="msg")
            nc.vector.tensor_tensor(
                out=msg[:, :ni].rearrange("p (b h x) -> p b h x", b=nt, h=H),
                in0=vgb[:, :ni].rearrange("p (b h x) -> p b h x", b=nt, h=H),
                in1=exbl[:, :nt * 8].rearrange("p (b h) -> p b h ()", b=nt)
                .broadcast_to([128, nt, H, HD]),
                op=mybir.AluOpType.mult)
            oh2b = p2.tile([128, B2 * 128], BF16, tag="oh2b")
            nc.vector.tensor_tensor(
                out=oh2b[:, :nt * 128].rearrange("p (b e) -> p b e", b=nt),
                in0=srcl2_s[:, t0:t0 + nt].rearrange("p b -> p b ()")
                .broadcast_to([128, nt, 128]),
                in1=iota128_s[:].rearrange("p e -> p () e")
                .broadcast_to([128, nt, 128]),
                op=mybir.AluOpType.is_equal)
            for j in range(nt):
                tj = t0 + j
                w = tj // T2W
                tt = tj % T2W
                if tt == 0:
                    aggT_cur[0] = p2a.tile([128, 128], F32, tag="aggT",
                                           name="aggT")
                aggT = aggT_cur[0]
                nc.tensor.matmul(out=aggT[:],
                                 lhsT=msg[:, j * 128:(j + 1) * 128],
                                 rhs=oh2b[:, j * 128:(j + 1) * 128],
                                 start=(tt == 0), stop=(tt == T2W - 1))
                if tt == T2W - 1:
                    aggT_sb = pD.tile([128, 128], BF16, tag="aggT_sb")
                    nc.vector.tensor_copy(out=aggT_sb[:], in_=aggT[:])
                    attn = p2p.tile([128, 128], F32, tag="attn")
                    nc.tensor.matmul(out=attn[:], lhsT=aggT_sb[:],
                                     rhs=wo_s[:], start=True, stop=False)
                    nc.tensor.matmul(out=attn[:], lhsT=ones_s[:],
                                     rhs=bo_s[:], start=False, stop=True)
                    xw2 = pD.tile([128, 128], F32, tag="xw2")
                    nc.sync.dma_start(xw2[:],
                                      t["x_own"][w * 128:(w + 1) * 128, :])
                    nc.vector.tensor_tensor(
                        out=x1_res[:, w * 128:(w + 1) * 128],
                        in0=xw2[:], in1=attn[:], op=mybir.AluOpType.add)

    # ---------------- Phase D ----------------
    with tc.tile_pool(name="pDm", bufs=1) as pDm, \
         tc.tile_pool(name="pDw", bufs=2) as pDw, \
         tc.tile_pool(name="pDp", bufs=2, space="PSUM") as pDp, \
         tc.tile_pool(name="pDh", bufs=2, space="PSUM") as pDh:
        x1v = x1_res[:].rearrange("p (w f) -> p w f", w=W)
        s1b = pDm.tile([128, W], F32, tag="s1b")
        nc.vector.tensor_reduce(out=s1b[:], in_=x1v, axis=mybir.AxisListType.X,
                                op=mybir.AluOpType.add)
        sqb = pDm.tile([128, W * 128], BF16, tag="sqb")
        nc.scalar.activation(out=sqb[:], in_=x1_res[:],
                             func=mybir.ActivationFunctionType.Square)
        s2b = pDm.tile([128, W], F32, tag="s2b")
        nc.vector.tensor_reduce(out=s2b[:],
                                in_=sqb[:].rearrange("p (w f) -> p w f", w=W),
                                axis=mybir.AxisListType.X,
                                op=mybir.AluOpType.add)
        mb = pDm.tile([128, W], F32, tag="mb")
        nc.vector.tensor_scalar_mul(mb[:], s1b[:], 1.0 / 128.0)
        m2b = pDm.tile([128, W], F32, tag="m2b")
        nc.vector.tensor_tensor(out=m2b[:], in0=mb[:], in1=mb[:],
                                op=mybir.AluOpType.mult)
        varb = pDm.tile([128, W], F32, tag="varb")
        nc.vector.scalar_tensor_tensor(out=varb[:], in0=s2b[:],
                                       scalar=1.0 / 128.0, in1=m2b[:],
                                       op0=mybir.AluOpType.mult,
                                       op1=mybir.AluOpType.subtract)
        stdb = pDm.tile([128, W], F32, tag="stdb")
        nc.scalar.activation(out=stdb[:], in_=varb[:],
                             func=mybir.ActivationFunctionType.Sqrt,
                             bias=eps_col[:])
        rstdb = pDm.tile([128, W], F32, tag="rstdb")
        nc.vector.reciprocal(out=rstdb[:], in_=stdb[:])
        negmb = pDm.tile([128, W], F32, tag="negmb")
        nc.vector.tensor_scalar_mul(negmb[:], mb[:], -1.0)
        x1t = pDm.tile([128, W * 128], BF16, tag="x1t")
        nc.vector.tensor_tensor(
            out=x1t[:].rearrange("p (w f) -> p w f", w=W), in0=x1v,
            in1=negmb[:].rearrange("p w -> p w ()").broadcast_to([128, W, 128]),
            op=mybir.AluOpType.add)
        x1n = pDm.tile([128, W * 128], BF16, tag="x1n")
        nc.vector.tensor_tensor(
            out=x1n[:].rearrange("p (w f) -> p w f", w=W),
            in0=x1t[:].rearrange("p (w f) -> p w f", w=W),
            in1=rstdb[:].rearrange("p w -> p w ()").broadcast_to([128, W, 128]),
            op=mybir.AluOpType.mult)

        for w in range(W):
            x1nT_ps = pDp.tile([128, 128], BF16, tag="x1nT_ps")
            nc.tensor.transpose(out=x1nT_ps[:],
                                in_=x1n[:, w * 128:(w + 1) * 128],
                                identity=ident_b[:])
            x1nT = pDw.tile([128, 128], BF16, tag="x1nT")
            nc.vector.tensor_copy(out=x1nT[:], in_=x1nT_ps[:])
            hsbT = pDw.tile([128, 4 * 128], BF16, tag="hsbT")
            for ch in range(4):
                hp = pDh.tile([128, 128], F32, tag="hp")
                nc.tensor.matmul(out=hp[:],
                                 lhsT=w1g_s[:, ch * 128:(ch + 1) * 128],
                                 rhs=x1nT[:], start=True, stop=True)
                nc.scalar.activation(out=hsbT[:, ch * 128:(ch + 1) * 128],
                                     in_=hp[:],
                                     func=mybir.ActivationFunctionType.Relu,
                                     bias=b12c_s[:, ch:ch + 1])
            ffn = pDp.tile([128, 128], F32, tag="ffn")
            for ch in range(4):
                nc.tensor.matmul(out=ffn[:],
                                 lhsT=hsbT[:, ch * 128:(ch + 1) * 128],
                                 rhs=w2_s[:, ch * 128:(ch + 1) * 128],
                                 start=(ch == 0), stop=False)
            nc.tensor.matmul(out=ffn[:], lhsT=ones_s[:], rhs=b2_s[:],
                             start=False, stop=True)
            outw = pDw.tile([128, 128], F32, tag="outw")
            nc.vector.tensor_tensor(out=outw[:],
                                    in0=x1_res[:, w * 128:(w + 1) * 128],
                                    in1=ffn[:], op=mybir.AluOpType.add)
            nc.sync.dma_start(t["out"][w * 128:(w + 1) * 128, :], outw[:])

    ctx.close()


def build_program(pp, nc_factory):
    import concourse.tile as tile
    nc = nc_factory()
    t = declare_io(nc, pp)
    with tile.TileContext(nc) as tc:
        build(tc, t, pp)
    nc.compile()
    return nc


# --------------------------------------------------------------------------
# Harness entry point
# --------------------------------------------------------------------------

NCORES = 8
W_PER_CORE = 49  # 8*49*128 = 50176 >= 50000 nodes


def _run_spmd_timed(nc, in_maps, n_cores, reps=4):
    """Execute the SPMD program via PJRT with device-staged inputs; returns
    (per-core results, estimated per-execution device ns)."""
    import time

    import jax
    from jax.experimental.shard_map import shard_map
    from jax.sharding import Mesh, NamedSharding, PartitionSpec

    from concourse.bass2jax import (_bass_exec_p, install_neuronx_cc_hook,
                                    partition_id_tensor)

    install_neuronx_cc_hook()
    partition_name = (nc.partition_id_tensor.name
                      if nc.partition_id_tensor else None)
    in_names, out_names, out_avals, zero_outs = [], [], [], []
    for alloc in nc.m.functions[0].allocations:
        if not isinstance(alloc, mybir.MemoryLocationSet):
            continue
        name = alloc.memorylocations[0].name
        if alloc.kind == "ExternalInput":
            if name != partition_name:
                in_names.append(name)
        elif alloc.kind == "ExternalOutput":
            shape = tuple(alloc.tensor_shape)
            dtype = mybir.dt.np(alloc.dtype)
            out_names.append(name)
            out_avals.append(jax.core.ShapedArray(shape, dtype))
            zero_outs.append(np.zeros(shape, dtype))
    n_params = len(in_names)
    n_outs = len(out_avals)
    in_names.extend(out_names)
    if partition_name is not None:
        in_names.append(partition_name)
    donate = tuple(range(n_params, n_params + n_outs))

    def _body(*args):
        operands = list(args)
        if partition_name is not None:
            operands.append(partition_id_tensor())
        outs = _bass_exec_p.bind(
            *operands, out_avals=tuple(out_avals), in_names=tuple(in_names),
            out_names=tuple(out_names), lowering_input_output_aliases=(),
            sim_require_finite=True, sim_require_nnan=True, nc=nc)
        return tuple(outs)

    devices = jax.devices()[:n_cores]
    mesh = Mesh(np.asarray(devices), ("core",))
    sharding = NamedSharding(mesh, PartitionSpec("core"))
    in_specs = (PartitionSpec("core"),) * (n_params + n_outs)
    out_specs = (PartitionSpec("core"),) * len(out_names)
    sharded = jax.jit(
        shard_map(_body, mesh=mesh, in_specs=in_specs, out_specs=out_specs,
                  check_rep=False),
        donate_argnums=donate, keep_unused=True)
    concat_in = [
        np.concatenate([np.asarray(in_maps[c][in_names[i]])
                        for c in range(n_cores)], axis=0)
        for i in range(n_params)]
    dev_in = [jax.device_put(a, sharding) for a in concat_in]

    def fresh_zeros():
        zs = [jax.device_put(
            np.zeros((n_cores * z.shape[0], *z.shape[1:]), z.dtype), sharding)
            for z in zero_outs]
        jax.block_until_ready(zs)
        return zs

    out_arrs = sharded(*dev_in, *fresh_zeros())
    jax.block_until_ready(out_arrs)
    results = [
        {name: np.asarray(out_arrs[i]).reshape(n_cores, *out_avals[i].shape)[c]
         for i, name in enumerate(out_names)}
        for c in range(n_cores)]
    if reps <= 0:
        return results, None

    # Amortized timing: the axon/PJRT dispatch round-trip is ~70-80 ms and
    # dominates a single-call wall measurement, but dispatch pipelines, so
    # chained executions expose the true per-execution device time as the
    # marginal cost. Chain by donating the previous call's output buffers
    # (the kernel fully overwrites every output) so device-side execution
    # is strictly serialized.
    def run_chain(k):
        zs = fresh_zeros()
        t0 = time.perf_counter()
        o = tuple(zs)
        for _ in range(k):
            o = sharded(*dev_in, *o)
        jax.block_until_ready(o)
        return time.perf_counter() - t0

    K = 32
    w1 = min(run_chain(1) for _ in range(max(reps, 2)))
    wk = min(run_chain(K) for _ in range(max(reps, 2)))
    marginal = (wk - w1) / (K - 1)
    best = max(marginal, 1e-6)
    return results, int(best * 1e9)


def kernel(**inputs):
    import sys
    if "/opt/trn_rl_repo" not in sys.path:
        sys.path.insert(0, "/opt/trn_rl_repo")
    import concourse.bacc as bacc

    x = np.asarray(inputs["x"], np.float32)
    edge_index = np.asarray(inputs["edge_index"])
    curv = np.asarray(inputs["curvature_embeddings"], np.float32)
    weights = {k: np.asarray(v) for k, v in inputs.items()
               if k not in ("x", "edge_index", "curvature_embeddings")}

    pp, in_maps = host_prep(x, edge_index, curv, weights, NCORES, W_PER_CORE)
    nc = build_program(pp, lambda: bacc.Bacc(
        "TRN2", target_bir_lowering=False, debug=False, num_devices=NCORES))
    results, best_ns = _run_spmd_timed(nc, in_maps, NCORES)
    kernel.last_exec_ns = best_ns
    out = np.concatenate([results[c]["out"] for c in range(NCORES)],
                         axis=0)[:x.shape[0]]
    return np.ascontiguousarray(out, dtype=np.float32)



# revision 62
# speedup vs baseline: 1.2301x; 1.2301x over previous
"""Curvphormer GNN layer as a Bass/Tile SPMD kernel for TRN2 (V6).

Design (per core c of NCORES, owning 49 windows x 128 nodes):
 - Edges sharded by src range. Pass 1 groups edges by 256-node tgt
   wide-windows (ww), sorted by src within a ww so pass-2 runs are
   contiguous. Pass 2 groups edges by own src-window as runs of R=4
   consecutive pass-1 slots (one indirect DMA per run-group reads ex).
 - Phase A: batched LN stats; per-window xn^T via PE transpose; q/k/v as
   bf16 matmuls to DRAM. AllGather(k) and AllGather(v) (both with Shared
   pair-HBM outputs); the k "hi" table is an offset view k_full[NCUT:]
   so int16 gather indices stay in range (no copy).
 - Pass 1 (super-blocks of 8 wws): q/k rows via batched NON-transposed
   dma_gather (slot-major, contiguous 256B writes — much faster on HW
   than transposed gathers); prod = q*k on DVE; per-head dot via strided
   DVE tensor_reduce; score = qk + PSUM(curv@4Wc + 4bc) added on DVE;
   batched exp(0.25*s). Denominator via half-one-hot rows (i128z, 256B)
   gathered once per super-block + lo/hi-masked ex matmuls into PSUM.
 - AllReduce(den, fp32, Shared out); full gathered v normalized LOCALLY
   on DVE (no second AllGather exposed between pass 1 and pass 2).
 - Pass 2 (32-tile blocks): vn gathered in node-pairs (idx=tgt//2, 512B
   elems) with parity folded into ex masks; ex via indirect run-gathers;
   aggregation via iota/is_equal one-hot matmuls into per-window PSUM;
   out = x + agg@Wo + FFN(LN2(x1)) with transposed-chunk FFN.
"""

import sys
if "/opt/trn_rl_repo" not in sys.path:
    sys.path.insert(0, "/opt/trn_rl_repo")

import numpy as np

import concourse.bass as bass
import concourse.mybir as mybir
from concourse.masks import make_identity

F32 = mybir.dt.float32
BF16 = mybir.dt.bfloat16
I32 = mybir.dt.int32
I16 = mybir.dt.int16

D = 128
H = 8
HD = 16
LN_EPS = 1e-5
R2 = 4          # pass-2 run length (ex rows per indirect read)
NCUT = 17408    # k_hi table base (68 wide-windows * 256)


def _bf(a):
    import ml_dtypes
    return np.asarray(a, np.float32).astype(ml_dtypes.bfloat16)


def _wrap16(flat, ncols):
    """int16 idx table [128, ncols]: slot s -> [s%16, s//16], replicated."""
    tab = np.zeros((128, ncols), np.int16)
    tab[:16] = np.asarray(flat, np.int16).reshape(-1, 16).T
    for r in range(16, 128, 16):
        tab[r:r + 16] = tab[:16]
    return tab


class P:
    def __init__(self, ncores, W, T1W, NG2W):
        self.ncores = ncores
        self.W = W
        self.T1W = T1W
        self.NWW = ncores * W // 2
        self.T1 = self.NWW * T1W
        self.NG2W = NG2W
        self.T2W = NG2W * R2
        self.T2 = W * self.T2W
        self.NG2 = W * NG2W
        self.nodes_pc = W * 128
        self.npad = ncores * W * 128


# --------------------------------------------------------------------------
# Host-side preprocessing
# --------------------------------------------------------------------------

def host_prep(x, edge_index, curv, weights, ncores, W):
    N = x.shape[0]
    E = edge_index.shape[1]
    nodes_pc = W * 128
    npad = ncores * nodes_pc
    assert npad >= N

    src = np.asarray(edge_index[0], dtype=np.int64)
    tgt = np.asarray(edge_index[1], dtype=np.int64)
    x_pad = np.zeros((npad, D), dtype=np.float32)
    x_pad[:N] = x

    core_of = (src // 128) // W
    order_by_core = np.argsort(core_of, kind="stable")
    counts = np.bincount(core_of, minlength=ncores)
    splits = np.split(order_by_core, np.cumsum(counts)[:-1])

    NWW = ncores * W // 2

    # ---- pass-1 slot assignment (per core), sorted by (ww, src) ----
    T1W = 0
    p1_orders = []
    for c in range(ncores):
        e_c = splits[c]
        ww_of = tgt[e_c] // 256
        order = np.lexsort((src[e_c], ww_of))
        e_sorted = e_c[order]
        p1_orders.append(e_sorted)
        cnt = np.bincount(tgt[e_sorted] // 256, minlength=NWW)
        T1W = max(T1W, int(np.ceil(cnt.max() / 128)))
    T1 = NWW * T1W
    S1 = T1 * 128

    # per-core pass-1 tables
    core_p1 = []
    for c in range(ncores):
        e_sorted = p1_orders[c]
        ww_sorted = tgt[e_sorted] // 256
        cnt = np.bincount(ww_sorted, minlength=NWW)
        starts = NWW and np.concatenate([[0], np.cumsum(cnt)[:-1]])
        slots = np.zeros(len(e_sorted), np.int64)
        for ww in range(NWW):
            k = cnt[ww]
            if k:
                sl = ww * T1W * 128 + np.arange(k)
                slots[starts[ww]:starts[ww] + k] = sl
        real1 = np.zeros(S1, bool)
        real1[slots] = True
        tgt1 = np.zeros(S1, np.int64)
        tgt1[slots] = tgt[e_sorted]
        src1 = np.zeros(S1, np.int64)
        src1[slots] = src[e_sorted]
        core_p1.append((e_sorted, slots, real1, tgt1, src1))

    # ---- pass-2 runs (per core, per window) ----
    NG2W = 0
    core_runs = []
    for c in range(ncores):
        e_sorted, slots, real1, tgt1, src1 = core_p1[c]
        w_loc = (src[e_sorted] // 128) - c * W
        runs_per_w = []
        for w in range(W):
            s_list = np.sort(slots[w_loc == w])
            if len(s_list) == 0:
                runs_per_w.append([])
                continue
            d = np.diff(s_list)
            segstart = np.concatenate([[0], np.flatnonzero(d != 1) + 1])
            seglen = np.diff(np.concatenate([segstart, [len(s_list)]]))
            runs = []
            for ss, ln in zip(segstart, seglen):
                for off in range(0, ln, R2):
                    runs.append((int(s_list[ss + off]),
                                 int(min(R2, ln - off))))
            runs_per_w.append(runs)
            NG2W = max(NG2W, (len(runs) + 127) // 128)
        core_runs.append(runs_per_w)

    pp = P(ncores, W, T1W, NG2W)
    T2, T2W, NG2 = pp.T2, pp.T2W, pp.NG2
    S2 = T2 * 128

    # ---- weights (common) ----
    g1, be1 = weights["g1"], weights["be1"]
    g2, be2 = weights["g2"], weights["be2"]

    def foldA(Wm, b):
        return (_bf(g1[:, None] * Wm),
                _bf((be1 @ Wm + b))[None, :])

    wqp, bqp = foldA(weights["Wq"], weights["bq"])
    wkp, bkp = foldA(weights["Wk"], weights["bk"])
    wvp, bvp = foldA(weights["Wv"], weights["bv"])
    w1g = _bf(g2[:, None] * weights["W1"])
    b12 = (be2 @ weights["W1"] + weights["b1"]).astype(np.float32)
    b12cols = np.ascontiguousarray(b12.reshape(4, 128).T)  # [128, 4]
    w2ch = _bf(np.ascontiguousarray(
        weights["W2"].astype(np.float32).reshape(4, 128, D)
        .transpose(1, 0, 2).reshape(128, 4 * D)))

    headmask = np.zeros((128, H), np.float32)
    for f in range(128):
        headmask[f, f // 16] = 1.0
    i128z = np.zeros((129, 128), np.float32)
    i128z[:128, :128] = np.eye(128)
    iota128 = np.tile(np.arange(128, dtype=np.float32)[None, :], (128, 1))

    common = {
        "wqp": wqp, "wkp": wkp, "wvp": wvp,
        "bqp": bqp, "bkp": bkp, "bvp": bvp,
        "wc4": _bf(4.0 * weights["Wc"]),
        "bc4": _bf(4.0 * weights["bc"])[None, :],
        "wo_b": _bf(weights["Wo"]), "bo_b": _bf(weights["bo"])[None, :],
        "w1g": w1g, "b12cols": b12cols.astype(np.float32),
        "w2ch": w2ch, "b2_b": _bf(weights["b2"])[None, :],
        "ones_b": np.ones((1, D), np.float32).astype(np.float32),
        "headmask": _bf(headmask),
        "i128z": _bf(i128z),
        "iota128_b": _bf(iota128),
    }
    common["ones_b"] = _bf(np.ones((1, D), np.float32))

    in_maps = []
    for c in range(ncores):
        e_sorted, slots, real1, tgt1, src1 = core_p1[c]

        qi = np.where(real1, src1 - c * nodes_pc, 0)
        klo = np.where(tgt1 < NCUT, tgt1, tgt1 - NCUT)
        ki = np.where(real1, klo, 0)
        ohm = np.where(real1, (tgt1 % 128).astype(np.float32), -1.0)
        ohmT = np.ascontiguousarray(ohm.reshape(T1, 128).T)
        is_lo = real1 & ((tgt1 % 256) < 128)
        is_hi = real1 & ((tgt1 % 256) >= 128)
        lobm = np.ascontiguousarray(
            is_lo.astype(np.float32).reshape(T1, 128).T)
        hibm = np.ascontiguousarray(
            is_hi.astype(np.float32).reshape(T1, 128).T)

        curv1 = np.zeros((S1, D), np.float32)
        curv1[slots] = curv[e_sorted]
        c1t = curv1.reshape(T1, 128, D).transpose(0, 2, 1)  # [T1, D, 128]
        curv1t = _bf(np.ascontiguousarray(
            c1t.reshape(T1 // 2, 2, D, 128).transpose(0, 2, 1, 3))
            .reshape((T1 // 2) * D, 256))

        # ---- pass 2 tables ----
        runs_per_w = core_runs[c]
        inv_slot1 = np.full(S1, -1, np.int64)
        inv_slot1[slots] = e_sorted
        vni = np.zeros(S2, np.int64)
        par = np.zeros(S2, np.float32)
        sl2 = np.full(S2, -1.0, np.float32)
        valid2 = np.zeros(S2, bool)
        exoff = np.zeros((128, NG2), np.int32)
        for w in range(W):
            runs = runs_per_w[w]
            for ri, (r0, ln) in enumerate(runs):
                g, p = ri // 128, ri % 128
                exoff[p, w * NG2W + g] = r0
                base_tile = (w * NG2W + g) * R2
                for t_ in range(ln):
                    e = inv_slot1[r0 + t_]
                    u = (base_tile + t_) * 128 + p
                    vni[u] = tgt[e] // 2
                    par[u] = float(tgt[e] % 2)
                    sl2[u] = float(src[e] - (c * W + w) * 128)
                    valid2[u] = True

        x_own = np.ascontiguousarray(x_pad[c * nodes_pc:(c + 1) * nodes_pc])

        m = dict(common)
        m.update({
            "x_own": x_own,
            "x_bf": _bf(x_own),
            "curv1t": curv1t,
            "lobm": _bf(lobm),
            "hibm": _bf(hibm),
            "qi16": _wrap16(qi, T1 * 8),
            "ki16": _wrap16(ki, T1 * 8),
            "ohm": _bf(ohmT),
            "vni16": _wrap16(vni, T2 * 8),
            "exoff": exoff,
            "srcl2": _bf(np.ascontiguousarray(
                sl2.reshape(T2, 128).T)),
            "blm": _bf(np.ascontiguousarray(
                (valid2 & (par == 0)).astype(np.float32).reshape(T2, 128).T)),
            "bhm": _bf(np.ascontiguousarray(
                (valid2 & (par == 1)).astype(np.float32).reshape(T2, 128).T)),
        })
        in_maps.append(m)

    return pp, in_maps


# --------------------------------------------------------------------------
# Device program
# --------------------------------------------------------------------------

def declare_io(nc, pp):
    t = {}

    def din(name, shape, dt=F32):
        t[name] = nc.dram_tensor(name, list(shape), dt, kind="ExternalInput").ap()

    W, T1, T2 = pp.W, pp.T1, pp.T2
    din("x_own", (pp.nodes_pc, D))
    din("x_bf", (pp.nodes_pc, D), BF16)
    din("curv1t", ((T1 // 2) * 128, 2 * D), BF16)
    din("lobm", (128, T1), BF16)
    din("hibm", (128, T1), BF16)
    din("qi16", (128, T1 * 8), I16)
    din("ki16", (128, T1 * 8), I16)
    din("ohm", (128, T1), BF16)
    din("vni16", (128, T2 * 8), I16)
    din("exoff", (128, pp.NG2), I32)
    din("srcl2", (128, T2), BF16)
    din("blm", (128, T2), BF16)
    din("bhm", (128, T2), BF16)
    for n, shp, dt in [
            ("wqp", (D, D), BF16), ("wkp", (D, D), BF16), ("wvp", (D, D), BF16),
            ("bqp", (1, D), BF16), ("bkp", (1, D), BF16), ("bvp", (1, D), BF16),
            ("wc4", (D, H), BF16), ("bc4", (1, H), BF16),
            ("wo_b", (D, D), BF16), ("bo_b", (1, D), BF16),
            ("w1g", (D, 4 * D), BF16), ("b12cols", (128, 4), F32),
            ("w2ch", (D, 4 * D), BF16), ("b2_b", (1, D), BF16),
            ("ones_b", (1, D), BF16), ("headmask", (D, H), BF16),
            ("i128z", (129, 128), BF16), ("iota128_b", (128, 128), BF16)]:
        din(n, shp, dt)
    t["out"] = nc.dram_tensor("out", [pp.nodes_pc, D], F32,
                              kind="ExternalOutput").ap()
    return t


def build(tc, t, pp):
    nc = tc.nc
    W, T1W, T1, T2W, T2, NWW = (pp.W, pp.T1W, pp.T1, pp.T2W, pp.T2,
                                pp.NWW)
    NW = pp.ncores * W
    rg = [list(range(pp.ncores))]
    from contextlib import ExitStack
    ctx = ExitStack()

    # internal DRAM
    q_own_d, _ = tc.tile([pp.nodes_pc, D], BF16, space="DRAM", name="q_own_d")
    k_own_d, _ = tc.tile([pp.nodes_pc, D], BF16, space="DRAM", name="k_own_d")
    v_own_d, _ = tc.tile([pp.nodes_pc, D], BF16, space="DRAM", name="v_own_d")
    shpool = ctx.enter_context(tc.tile_pool(name="shdram", space="DRAM",
                                            bufs=1))
    k_full = shpool.tile([pp.npad, D], BF16, name="k_full",
                         addr_space="Shared")
    v_full = shpool.tile([pp.npad, D], BF16, name="v_full",
                         addr_space="Shared")
    den_d, _ = tc.tile([NW * 128, H], F32, space="DRAM", name="den_d")
    den_all = shpool.tile([NW * 128, H], F32, name="den_all",
                          addr_space="Shared")
    vn_full, _ = tc.tile([pp.npad, D], BF16, space="DRAM", name="vn_full")
    ex_d2, _ = tc.tile([T1 * 128 + 8, H], BF16, space="DRAM", name="ex_d2")

    const = ctx.enter_context(tc.tile_pool(name="const", bufs=1))

    def load_const(name):
        ap = t[name]
        tl = const.tile(list(ap.shape), ap.dtype, name=f"c_{name}")
        nc.sync.dma_start(tl[:], ap[:])
        return tl

    wqp_s = load_const("wqp"); wkp_s = load_const("wkp"); wvp_s = load_const("wvp")
    bqp_s = load_const("bqp"); bkp_s = load_const("bkp"); bvp_s = load_const("bvp")
    wc4_s = load_const("wc4"); bc4_s = load_const("bc4")
    wo_s = load_const("wo_b"); bo_s = load_const("bo_b")
    w1g_s = load_const("w1g"); b12c_s = load_const("b12cols")
    w2_s = load_const("w2ch"); b2_s = load_const("b2_b")
    ones_s = load_const("ones_b"); hmask_s = load_const("headmask")
    iota128_s = load_const("iota128_b")

    ident = const.tile([128, 128], F32, name="ident")
    make_identity(nc, ident[:])
    ident_b = const.tile([128, 128], BF16, name="ident_b")
    nc.vector.tensor_copy(out=ident_b[:], in_=ident[:])
    eps_col = const.tile([128, 1], F32, name="eps_col")
    nc.vector.memset(eps_col[:], LN_EPS)
    zrow = const.tile([8, H], BF16, name="zrow")
    nc.vector.memset(zrow[:], 0.0)

    # residents
    den_tab = const.tile([128, NWW * 2 * H], F32, name="den_tab")
    x1_res = const.tile([128, W * 128], F32, name="x1_res")

    # ---------------- Phase A ----------------
    with tc.tile_pool(name="pA", bufs=1) as pA, \
         tc.tile_pool(name="pAw", bufs=2) as pAw, \
         tc.tile_pool(name="pAp", bufs=2, space="PSUM") as pAp:
        xb = pA.tile([128, W * 128], BF16, tag="xb")
        nc.sync.dma_start(
            xb[:].rearrange("p (w f) -> p w f", w=W),
            t["x_bf"][:].rearrange("(w p) f -> p w f", p=128))
        xv = xb[:].rearrange("p (w f) -> p w f", w=W)
        s1 = pA.tile([128, W], F32, tag="s1")
        nc.vector.tensor_reduce(out=s1[:], in_=xv, axis=mybir.AxisListType.X,
                                op=mybir.AluOpType.add)
        sq = pA.tile([128, W * 128], BF16, tag="sq")
        nc.scalar.activation(out=sq[:], in_=xb[:],
                             func=mybir.ActivationFunctionType.Square)
        s2 = pA.tile([128, W], F32, tag="s2")
        nc.vector.tensor_reduce(out=s2[:],
                                in_=sq[:].rearrange("p (w f) -> p w f", w=W),
                                axis=mybir.AxisListType.X,
                                op=mybir.AluOpType.add)
        mcol = pA.tile([128, W], F32, tag="mcol")
        nc.vector.tensor_scalar_mul(mcol[:], s1[:], 1.0 / 128.0)
        m2c = pA.tile([128, W], F32, tag="m2c")
        nc.vector.tensor_tensor(out=m2c[:], in0=mcol[:], in1=mcol[:],
                                op=mybir.AluOpType.mult)
        var = pA.tile([128, W], F32, tag="var")
        nc.vector.scalar_tensor_tensor(out=var[:], in0=s2[:],
                                       scalar=1.0 / 128.0, in1=m2c[:],
                                       op0=mybir.AluOpType.mult,
                                       op1=mybir.AluOpType.subtract)
        stdc = pA.tile([128, W], F32, tag="stdc")
        nc.scalar.activation(out=stdc[:], in_=var[:],
                             func=mybir.ActivationFunctionType.Sqrt,
                             bias=eps_col[:])
        rstd = pA.tile([128, W], F32, tag="rstd")
        nc.vector.reciprocal(out=rstd[:], in_=stdc[:])
        negm = pA.tile([128, W], F32, tag="negm")
        nc.vector.tensor_scalar_mul(negm[:], mcol[:], -1.0)
        xnt = pA.tile([128, W * 128], BF16, tag="xnt")
        nc.vector.tensor_tensor(
            out=xnt[:].rearrange("p (w f) -> p w f", w=W), in0=xv,
            in1=negm[:].rearrange("p w -> p w ()").broadcast_to([128, W, 128]),
            op=mybir.AluOpType.add)
        xn = pA.tile([128, W * 128], BF16, tag="xn")
        nc.vector.tensor_tensor(
            out=xn[:].rearrange("p (w f) -> p w f", w=W),
            in0=xnt[:].rearrange("p (w f) -> p w f", w=W),
            in1=rstd[:].rearrange("p w -> p w ()").broadcast_to([128, W, 128]),
            op=mybir.AluOpType.mult)

        for w in range(W):
            xnT_ps = pAp.tile([128, 128], BF16, tag="xnT_ps")
            nc.tensor.transpose(out=xnT_ps[:], in_=xn[:, w * 128:(w + 1) * 128],
                                identity=ident_b[:])
            xnT = pAw.tile([128, 128], BF16, tag="xnT")
            nc.vector.tensor_copy(out=xnT[:], in_=xnT_ps[:])
            for nm, wmat, brow, dst in (
                    ("q", wqp_s, bqp_s, q_own_d), ("k", wkp_s, bkp_s, k_own_d),
                    ("v", wvp_s, bvp_s, v_own_d)):
                ps = pAp.tile([128, 128], F32, tag="ps")
                nc.tensor.matmul(out=ps[:], lhsT=xnT[:], rhs=wmat[:],
                                 start=True, stop=False)
                nc.tensor.matmul(out=ps[:], lhsT=ones_s[:], rhs=brow[:],
                                 start=False, stop=True)
                ot = pAw.tile([128, 128], BF16, tag=f"o_{nm}")
                nc.scalar.activation(out=ot[:], in_=ps[:],
                                     func=mybir.ActivationFunctionType.Copy)
                nc.sync.dma_start(dst[w * 128:(w + 1) * 128, :], ot[:])

    nc.gpsimd.collective_compute(
        "AllGather", mybir.AluOpType.bypass, replica_groups=rg,
        ins=[k_own_d.opt()], outs=[k_full.opt()])
    nc.gpsimd.collective_compute(
        "AllGather", mybir.AluOpType.bypass, replica_groups=rg,
        ins=[v_own_d.opt()], outs=[v_full.opt()])

    # ---------------- Pass 1 ----------------
    NBLK = NWW // 2
    nt1 = 2 * T1W
    BB = 4                        # blocks per super-block
    NSB = (NBLK + BB - 1) // BB
    WWCUT = NCUT // 256           # first hi wide-window
    k_hi_v = k_full[NCUT:pp.npad, :]
    with tc.tile_pool(name="p1", bufs=2) as p1, \
         tc.tile_pool(name="p1p", bufs=2, space="PSUM") as p1p, \
         tc.tile_pool(name="p1d", bufs=2, space="PSUM") as p1d:
        nc.sync.dma_start(ex_d2[T1 * 128:T1 * 128 + 8, :], zrow[:])
        for sbi in range(NSB):
            b0 = sbi * BB
            nb = min(BB, NBLK - b0)
            nt = nb * nt1
            t0 = b0 * nt1
            s0 = t0 * 128
            ni = nt * 128
            cvb = p1.tile([128, BB * nt1 * 128], BF16, tag="cvb")
            nc.sync.dma_start(
                cvb[:, :ni].rearrange("p (b e) -> p b e", e=256),
                t["curv1t"][(t0 // 2) * 128:((t0 + nt) // 2) * 128, :]
                .rearrange("(b p) e -> p b e", p=128))
            qi_s = p1.tile([128, BB * nt1 * 8], I16, tag="qi_s")
            nc.sync.dma_start(qi_s[:, :nt * 8],
                              t["qi16"][:, s0 // 16:(s0 + ni) // 16])
            ki_s = p1.tile([128, BB * nt1 * 8], I16, tag="ki_s")
            nc.sync.dma_start(ki_s[:, :nt * 8],
                              t["ki16"][:, s0 // 16:(s0 + ni) // 16])
            ohm_s = p1.tile([128, BB * nt1], BF16, tag="ohm_s")
            nc.sync.dma_start(ohm_s[:, :nt], t["ohm"][:, t0:t0 + nt])
            lob_s = p1.tile([128, BB * nt1], BF16, tag="lob_s")
            nc.sync.dma_start(lob_s[:, :nt], t["lobm"][:, t0:t0 + nt])
            hib_s = p1.tile([128, BB * nt1], BF16, tag="hib_s")
            nc.sync.dma_start(hib_s[:, :nt], t["hibm"][:, t0:t0 + nt])
            qT = p1.tile([128, BB * nt1 * 128], BF16, tag="qT")
            nc.gpsimd.dma_gather(
                out_ap=qT[:, :ni].rearrange("p (i e) -> p i e", i=nt),
                in_ap=q_own_d[:], idxs_ap=qi_s[:, :ni // 16],
                num_idxs=ni, num_idxs_reg=ni, elem_size=128,
                single_packet=False)
            kT = p1.tile([128, BB * nt1 * 128], BF16, tag="kT")
            ww_lo, ww_hi = 2 * b0, 2 * (b0 + nb)
            segs = []
            if ww_lo < WWCUT:
                segs.append((ww_lo, min(ww_hi, WWCUT), k_full[:]))
            if ww_hi > WWCUT:
                segs.append((max(ww_lo, WWCUT), ww_hi, k_hi_v))
            for (wa, wb, ktab_ap) in segs:
                ta = (wa - ww_lo) * T1W
                tb = (wb - ww_lo) * T1W
                na = (tb - ta) * 128
                nc.gpsimd.dma_gather(
                    out_ap=kT[:, ta * 128:tb * 128]
                    .rearrange("p (i e) -> p i e", i=tb - ta),
                    in_ap=ktab_ap,
                    idxs_ap=ki_s[:, ta * 8:tb * 8],
                    num_idxs=na, num_idxs_reg=na, elem_size=128,
                    single_packet=False)
            ohb = p1.tile([128, BB * nt1 * 128], BF16, tag="ohb")
            nc.vector.tensor_tensor(
                out=ohb[:, :ni].rearrange("p (i e) -> p i e", i=nt),
                in0=ohm_s[:, :nt].rearrange("p i -> p i ()")
                .broadcast_to([128, nt, 128]),
                in1=iota128_s[:].rearrange("p e -> p () e")
                .broadcast_to([128, nt, 128]),
                op=mybir.AluOpType.is_equal)
            prodT = p1.tile([128, BB * nt1 * 128], BF16, tag="prodT")
            nc.vector.tensor_tensor(out=prodT[:, :ni], in0=qT[:, :ni],
                                    in1=kT[:, :ni], op=mybir.AluOpType.mult)
            qkred = p1.tile([128, BB * nt1 * 8], F32, tag="qkred")
            nc.vector.tensor_reduce(
                out=qkred[:, :nt * 8].rearrange("p (b h) -> p b h", h=H),
                in_=prodT[:, :ni].rearrange("p (b h x) -> p b h x",
                                            b=nt, h=H),
                axis=mybir.AxisListType.X, op=mybir.AluOpType.add)
            exb = p1.tile([128, BB * nt1 * 8], BF16, tag="exb")
            for bl in range(nb):
                sc_ps = p1p.tile([128, nt1 * 8], F32, tag="sc_ps")
                for j0 in range(nt1):
                    j = bl * nt1 + j0
                    scj = sc_ps[:, j0 * 8:(j0 + 1) * 8]
                    nc.tensor.matmul(out=scj,
                                     lhsT=cvb[:, j * 128:(j + 1) * 128],
                                     rhs=wc4_s[:], start=True, stop=False)
                    nc.tensor.matmul(out=scj, lhsT=ones_s[:], rhs=bc4_s[:],
                                     start=False, stop=True)
                s_sb = p1.tile([128, nt1 * 8], F32, tag="s_sb")
                nc.vector.tensor_tensor(
                    out=s_sb[:],
                    in0=qkred[:, bl * nt1 * 8:(bl + 1) * nt1 * 8],
                    in1=sc_ps[:], op=mybir.AluOpType.add)
                nc.scalar.activation(
                    out=exb[:, bl * nt1 * 8:(bl + 1) * nt1 * 8],
                    in_=s_sb[:], func=mybir.ActivationFunctionType.Exp,
                    scale=0.25)
            exl = p1.tile([128, BB * nt1 * 8], BF16, tag="exl")
            exh = p1.tile([128, BB * nt1 * 8], BF16, tag="exh")
            nc.vector.tensor_tensor(
                out=exl[:, :nt * 8].rearrange("p (b h) -> p b h", h=H),
                in0=exb[:, :nt * 8].rearrange("p (b h) -> p b h", h=H),
                in1=lob_s[:, :nt].rearrange("p b -> p b ()")
                .broadcast_to([128, nt, H]),
                op=mybir.AluOpType.mult)
            nc.vector.tensor_tensor(
                out=exh[:, :nt * 8].rearrange("p (b h) -> p b h", h=H),
                in0=exb[:, :nt * 8].rearrange("p (b h) -> p b h", h=H),
                in1=hib_s[:, :nt].rearrange("p b -> p b ()")
                .broadcast_to([128, nt, H]),
                op=mybir.AluOpType.mult)
            ohv = ohb[:, :ni].rearrange("p (i e) -> p i e", i=nt)
            for bl in range(nb):
                for i in range(2):
                    ww = 2 * (b0 + bl) + i
                    psd_lo = p1d.tile([128, H], F32, tag="psd_lo",
                                      name="psd_lo")
                    psd_hi = p1d.tile([128, H], F32, tag="psd_hi",
                                      name="psd_hi")
                    for tt in range(T1W):
                        jj = bl * nt1 + i * T1W + tt
                        nc.tensor.matmul(out=psd_lo[:], lhsT=ohv[:, jj, :],
                                         rhs=exl[:, jj * 8:(jj + 1) * 8],
                                         start=(tt == 0),
                                         stop=(tt == T1W - 1))
                        nc.tensor.matmul(out=psd_hi[:], lhsT=ohv[:, jj, :],
                                         rhs=exh[:, jj * 8:(jj + 1) * 8],
                                         start=(tt == 0),
                                         stop=(tt == T1W - 1))
                    nc.vector.tensor_copy(
                        out=den_tab[:, ww * 2 * H:ww * 2 * H + H],
                        in_=psd_lo[:])
                    nc.vector.tensor_copy(
                        out=den_tab[:, ww * 2 * H + H:(ww + 1) * 2 * H],
                        in_=psd_hi[:])
            nc.sync.dma_start(
                ex_d2[s0:s0 + ni, :].rearrange("(b p) h -> p b h", p=128),
                exb[:, :nt * 8].rearrange("p (b h) -> p b h", h=H))

        nc.sync.dma_start(
            den_d[:].rearrange("(w p) h -> p w h", p=128),
            den_tab[:].rearrange("p (w h) -> p w h", h=H))

    nc.gpsimd.collective_compute(
        "AllReduce", mybir.AluOpType.add, replica_groups=rg,
        ins=[den_d.opt()], outs=[den_all.opt()])

    # ---------------- Phase C: normalize full gathered v locally ---------
    with tc.tile_pool(name="pC", bufs=2) as pC:
        CHV = 8192
        nchv = (pp.npad + CHV - 1) // CHV
        for ch in range(nchv):
            r0 = ch * CHV
            nr = min(CHV, pp.npad - r0)
            na = nr // 128
            vb = pC.tile([128, (CHV // 128) * 128], BF16, tag="vb")
            nc.sync.dma_start(
                vb[:, :na * 128].rearrange("p (a f) -> p a f", a=na),
                v_full[r0:r0 + nr, :].rearrange("(a p) f -> p a f", p=128))
            db = pC.tile([128, (CHV // 128) * H], F32, tag="db")
            nc.sync.dma_start(
                db[:, :na * H].rearrange("p (a h) -> p a h", a=na),
                den_all[r0:r0 + nr, :].rearrange("(a p) h -> p a h", p=128))
            nc.vector.tensor_scalar_max(db[:, :na * H], db[:, :na * H],
                                        1e-30)
            rec = pC.tile([128, (CHV // 128) * H], F32, tag="rec")
            nc.vector.reciprocal(out=rec[:, :na * H], in_=db[:, :na * H])
            vnb = pC.tile([128, (CHV // 128) * 128], BF16, tag="vnb")
            nc.vector.tensor_tensor(
                out=vnb[:, :na * 128]
                .rearrange("p (a h x) -> p a h x", a=na, h=H),
                in0=vb[:, :na * 128]
                .rearrange("p (a h x) -> p a h x", a=na, h=H),
                in1=rec[:, :na * H].rearrange("p (a h) -> p a h ()", a=na)
                .broadcast_to([128, na, H, HD]),
                op=mybir.AluOpType.mult)
            nc.sync.dma_start(
                vn_full[r0:r0 + nr, :].rearrange("(a p) f -> p a f", p=128),
                vnb[:, :na * 128].rearrange("p (a f) -> p a f", a=na))

    # ---------------- Pass 2 ----------------
    B2 = 32
    NB2 = (T2 + B2 - 1) // B2
    vn_pair = vn_full[:].rearrange("(a b) f -> a (b f)", b=2)
    with tc.tile_pool(name="p2", bufs=2) as p2, \
         tc.tile_pool(name="p2c", bufs=1) as p2c, \
         tc.tile_pool(name="p2p", bufs=2, space="PSUM") as p2p, \
         tc.tile_pool(name="p2a", bufs=2, space="PSUM") as p2a, \
         tc.tile_pool(name="pD", bufs=2) as pD:
        vni_s = p2c.tile([128, T2 * 8], I16, name="vni_s")
        nc.sync.dma_start(vni_s[:], t["vni16"][:])
        exoff_s = p2c.tile([128, pp.NG2], I32, name="exoff_s")
        nc.sync.dma_start(exoff_s[:], t["exoff"][:])
        srcl2_s = p2c.tile([128, T2], BF16, name="srcl2_s")
        nc.sync.dma_start(srcl2_s[:], t["srcl2"][:])
        blm_s = p2c.tile([128, T2], BF16, name="blm_s")
        nc.sync.dma_start(blm_s[:], t["blm"][:])
        bhm_s = p2c.tile([128, T2], BF16, name="bhm_s")
        nc.sync.dma_start(bhm_s[:], t["bhm"][:])

        aggT_cur = [None]
        for bi in range(NB2):
            t0 = bi * B2
            nt = min(B2, T2 - t0)
            s0 = t0 * 128
            ni = nt * 128
            vgbp = p2.tile([128, B2 * 256], BF16, tag="vgbp")
            nc.gpsimd.dma_gather(
                out_ap=vgbp[:, :nt * 256].rearrange("p (i e) -> p i e", i=nt),
                in_ap=vn_pair, idxs_ap=vni_s[:, s0 // 16:(s0 + ni) // 16],
                num_idxs=ni, num_idxs_reg=ni, elem_size=256,
                single_packet=False)
            egb = p2.tile([128, B2 * 8], BF16, tag="egb")
            ng = (nt + R2 - 1) // R2
            for gi in range(ng):
                g = t0 // R2 + gi
                nc.gpsimd.indirect_dma_start(
                    out=egb[:, gi * R2 * 8:(gi + 1) * R2 * 8],
                    out_offset=None,
                    in_=ex_d2[:],
                    in_offset=bass.IndirectOffsetOnAxis(
                        ap=exoff_s[:, g:g + 1], axis=0))
            exbl = p2.tile([128, B2 * 8], BF16, tag="exbl")
            nc.vector.tensor_tensor(
                out=exbl[:, :nt * 8].rearrange("p (b h) -> p b h", b=nt),
                in0=egb[:, :nt * 8].rearrange("p (b h) -> p b h", b=nt),
                in1=blm_s[:, t0:t0 + nt].rearrange("p b -> p b ()")
                .broadcast_to([128, nt, H]),
                op=mybir.AluOpType.mult)
            exbh = p2.tile([128, B2 * 8], BF16, tag="exbh")
            nc.vector.tensor_tensor(
                out=exbh[:, :nt * 8].rearrange("p (b h) -> p b h", b=nt),
                in0=egb[:, :nt * 8].rearrange("p (b h) -> p b h", b=nt),
                in1=bhm_s[:, t0:t0 + nt].rearrange("p b -> p b ()")
                .broadcast_to([128, nt, H]),
                op=mybir.AluOpType.mult)
            vv = vgbp[:].rearrange("p (i e) -> p i e", i=B2)
            msglo = p2.tile([128, B2 * 128], BF16, tag="msglo")
            nc.vector.tensor_tensor(
                out=msglo[:, :nt * 128]
                .rearrange("p (b h x) -> p b h x", b=nt, h=H),
                in0=vv[:, :nt, 0:128].rearrange("p b (h x) -> p b h x", h=H),
                in1=exbl[:, :nt * 8].rearrange("p (b h) -> p b h ()", b=nt)
                .broadcast_to([128, nt, H, HD]),
                op=mybir.AluOpType.mult)
            msghi = p2.tile([128, B2 * 128], BF16, tag="msghi")
            nc.vector.tensor_tensor(
                out=msghi[:, :nt * 128]
                .rearrange("p (b h x) -> p b h x", b=nt, h=H),
                in0=vv[:, :nt, 128:256].rearrange("p b (h x) -> p b h x", h=H),
                in1=exbh[:, :nt * 8].rearrange("p (b h) -> p b h ()", b=nt)
                .broadcast_to([128, nt, H, HD]),
                op=mybir.AluOpType.mult)
            oh2b = p2.tile([128, B2 * 128], BF16, tag="oh2b")
            nc.vector.tensor_tensor(
                out=oh2b[:, :nt * 128].rearrange("p (b e) -> p b e", b=nt),
                in0=srcl2_s[:, t0:t0 + nt].rearrange("p b -> p b ()")
                .broadcast_to([128, nt, 128]),
                in1=iota128_s[:].rearrange("p e -> p () e")
                .broadcast_to([128, nt, 128]),
                op=mybir.AluOpType.is_equal)
            for j in range(nt):
                tj = t0 + j
                w = tj // T2W
                tt = tj % T2W
                if tt == 0:
                    aggT_cur[0] = p2a.tile([128, 128], F32, tag="aggT",
                                           name="aggT")
                aggT = aggT_cur[0]
                nc.tensor.matmul(out=aggT[:],
                                 lhsT=msglo[:, j * 128:(j + 1) * 128],
                                 rhs=oh2b[:, j * 128:(j + 1) * 128],
                                 start=(tt == 0), stop=False)
                nc.tensor.matmul(out=aggT[:],
                                 lhsT=msghi[:, j * 128:(j + 1) * 128],
                                 rhs=oh2b[:, j * 128:(j + 1) * 128],
                                 start=False, stop=(tt == T2W - 1))
                if tt == T2W - 1:
                    aggT_sb = pD.tile([128, 128], BF16, tag="aggT_sb")
                    nc.vector.tensor_copy(out=aggT_sb[:], in_=aggT[:])
                    attn = p2p.tile([128, 128], F32, tag="attn")
                    nc.tensor.matmul(out=attn[:], lhsT=aggT_sb[:],
                                     rhs=wo_s[:], start=True, stop=False)
                    nc.tensor.matmul(out=attn[:], lhsT=ones_s[:],
                                     rhs=bo_s[:], start=False, stop=True)
                    xw2 = pD.tile([128, 128], F32, tag="xw2")
                    nc.sync.dma_start(xw2[:],
                                      t["x_own"][w * 128:(w + 1) * 128, :])
                    nc.vector.tensor_tensor(
                        out=x1_res[:, w * 128:(w + 1) * 128],
                        in0=xw2[:], in1=attn[:], op=mybir.AluOpType.add)

    # ---------------- Phase D ----------------
    with tc.tile_pool(name="pDm", bufs=1) as pDm, \
         tc.tile_pool(name="pDw", bufs=2) as pDw, \
         tc.tile_pool(name="pDp", bufs=2, space="PSUM") as pDp, \
         tc.tile_pool(name="pDh", bufs=2, space="PSUM") as pDh:
        x1v = x1_res[:].rearrange("p (w f) -> p w f", w=W)
        s1b = pDm.tile([128, W], F32, tag="s1b")
        nc.vector.tensor_reduce(out=s1b[:], in_=x1v, axis=mybir.AxisListType.X,
                                op=mybir.AluOpType.add)
        sqb = pDm.tile([128, W * 128], BF16, tag="sqb")
        nc.scalar.activation(out=sqb[:], in_=x1_res[:],
                             func=mybir.ActivationFunctionType.Square)
        s2b = pDm.tile([128, W], F32, tag="s2b")
        nc.vector.tensor_reduce(out=s2b[:],
                                in_=sqb[:].rearrange("p (w f) -> p w f", w=W),
                                axis=mybir.AxisListType.X,
                                op=mybir.AluOpType.add)
        mb = pDm.tile([128, W], F32, tag="mb")
        nc.vector.tensor_scalar_mul(mb[:], s1b[:], 1.0 / 128.0)
        m2b = pDm.tile([128, W], F32, tag="m2b")
        nc.vector.tensor_tensor(out=m2b[:], in0=mb[:], in1=mb[:],
                                op=mybir.AluOpType.mult)
        varb = pDm.tile([128, W], F32, tag="varb")
        nc.vector.scalar_tensor_tensor(out=varb[:], in0=s2b[:],
                                       scalar=1.0 / 128.0, in1=m2b[:],
                                       op0=mybir.AluOpType.mult,
                                       op1=mybir.AluOpType.subtract)
        stdb = pDm.tile([128, W], F32, tag="stdb")
        nc.scalar.activation(out=stdb[:], in_=varb[:],
                             func=mybir.ActivationFunctionType.Sqrt,
                             bias=eps_col[:])
        rstdb = pDm.tile([128, W], F32, tag="rstdb")
        nc.vector.reciprocal(out=rstdb[:], in_=stdb[:])
        negmb = pDm.tile([128, W], F32, tag="negmb")
        nc.vector.tensor_scalar_mul(negmb[:], mb[:], -1.0)
        x1t = pDm.tile([128, W * 128], BF16, tag="x1t")
        nc.vector.tensor_tensor(
            out=x1t[:].rearrange("p (w f) -> p w f", w=W), in0=x1v,
            in1=negmb[:].rearrange("p w -> p w ()").broadcast_to([128, W, 128]),
            op=mybir.AluOpType.add)
        x1n = pDm.tile([128, W * 128], BF16, tag="x1n")
        nc.vector.tensor_tensor(
            out=x1n[:].rearrange("p (w f) -> p w f", w=W),
            in0=x1t[:].rearrange("p (w f) -> p w f", w=W),
            in1=rstdb[:].rearrange("p w -> p w ()").broadcast_to([128, W, 128]),
            op=mybir.AluOpType.mult)

        for w in range(W):
            x1nT_ps = pDp.tile([128, 128], BF16, tag="x1nT_ps")
            nc.tensor.transpose(out=x1nT_ps[:],
                                in_=x1n[:, w * 128:(w + 1) * 128],
                                identity=ident_b[:])
            x1nT = pDw.tile([128, 128], BF16, tag="x1nT")
            nc.vector.tensor_copy(out=x1nT[:], in_=x1nT_ps[:])
            hsbT = pDw.tile([128, 4 * 128], BF16, tag="hsbT")
            for ch in range(4):
                hp = pDh.tile([128, 128], F32, tag="hp")
                nc.tensor.matmul(out=hp[:],
                                 lhsT=w1g_s[:, ch * 128:(ch + 1) * 128],
                                 rhs=x1nT[:], start=True, stop=True)
                nc.scalar.activation(out=hsbT[:, ch * 128:(ch + 1) * 128],
                                     in_=hp[:],
                                     func=mybir.ActivationFunctionType.Relu,
                                     bias=b12c_s[:, ch:ch + 1])
            ffn = pDp.tile([128, 128], F32, tag="ffn")
            for ch in range(4):
                nc.tensor.matmul(out=ffn[:],
                                 lhsT=hsbT[:, ch * 128:(ch + 1) * 128],
                                 rhs=w2_s[:, ch * 128:(ch + 1) * 128],
                                 start=(ch == 0), stop=False)
            nc.tensor.matmul(out=ffn[:], lhsT=ones_s[:], rhs=b2_s[:],
                             start=False, stop=True)
            outw = pDw.tile([128, 128], F32, tag="outw")
            nc.vector.tensor_tensor(out=outw[:],
                                    in0=x1_res[:, w * 128:(w + 1) * 128],
                                    in1=ffn[:], op=mybir.AluOpType.add)
            nc.sync.dma_start(t["out"][w * 128:(w + 1) * 128, :], outw[:])

    ctx.close()


def build_program(pp, nc_factory):
    import concourse.tile as tile
    nc = nc_factory()
    t = declare_io(nc, pp)
    with tile.TileContext(nc) as tc:
        build(tc, t, pp)
    nc.compile()
    return nc


# --------------------------------------------------------------------------
# Harness entry point
# --------------------------------------------------------------------------

NCORES = 8
W_PER_CORE = 49  # 8*49*128 = 50176 >= 50000 nodes


def _run_spmd_timed(nc, in_maps, n_cores, reps=4):
    """Execute the SPMD program via PJRT with device-staged inputs; returns
    (per-core results, estimated per-execution device ns)."""
    import time

    import jax
    from jax.experimental.shard_map import shard_map
    from jax.sharding import Mesh, NamedSharding, PartitionSpec

    from concourse.bass2jax import (_bass_exec_p, install_neuronx_cc_hook,
                                    partition_id_tensor)

    install_neuronx_cc_hook()
    partition_name = (nc.partition_id_tensor.name
                      if nc.partition_id_tensor else None)
    in_names, out_names, out_avals, zero_outs = [], [], [], []
    for alloc in nc.m.functions[0].allocations:
        if not isinstance(alloc, mybir.MemoryLocationSet):
            continue
        name = alloc.memorylocations[0].name
        if alloc.kind == "ExternalInput":
            if name != partition_name:
                in_names.append(name)
        elif alloc.kind == "ExternalOutput":
            shape = tuple(alloc.tensor_shape)
            dtype = mybir.dt.np(alloc.dtype)
            out_names.append(name)
            out_avals.append(jax.core.ShapedArray(shape, dtype))
            zero_outs.append(np.zeros(shape, dtype))
    n_params = len(in_names)
    n_outs = len(out_avals)
    in_names.extend(out_names)
    if partition_name is not None:
        in_names.append(partition_name)
    donate = tuple(range(n_params, n_params + n_outs))

    def _body(*args):
        operands = list(args)
        if partition_name is not None:
            operands.append(partition_id_tensor())
        outs = _bass_exec_p.bind(
            *operands, out_avals=tuple(out_avals), in_names=tuple(in_names),
            out_names=tuple(out_names), lowering_input_output_aliases=(),
            sim_require_finite=True, sim_require_nnan=True, nc=nc)
        return tuple(outs)

    devices = jax.devices()[:n_cores]
    mesh = Mesh(np.asarray(devices), ("core",))
    sharding = NamedSharding(mesh, PartitionSpec("core"))
    in_specs = (PartitionSpec("core"),) * (n_params + n_outs)
    out_specs = (PartitionSpec("core"),) * len(out_names)
    sharded = jax.jit(
        shard_map(_body, mesh=mesh, in_specs=in_specs, out_specs=out_specs,
                  check_rep=False),
        donate_argnums=donate, keep_unused=True)
    concat_in = [
        np.concatenate([np.asarray(in_maps[c][in_names[i]])
                        for c in range(n_cores)], axis=0)
        for i in range(n_params)]
    dev_in = [jax.device_put(a, sharding) for a in concat_in]

    def fresh_zeros():
        zs = [jax.device_put(
            np.zeros((n_cores * z.shape[0], *z.shape[1:]), z.dtype), sharding)
            for z in zero_outs]
        jax.block_until_ready(zs)
        return zs

    out_arrs = sharded(*dev_in, *fresh_zeros())
    jax.block_until_ready(out_arrs)
    results = [
        {name: np.asarray(out_arrs[i]).reshape(n_cores, *out_avals[i].shape)[c]
         for i, name in enumerate(out_names)}
        for c in range(n_cores)]
    if reps <= 0:
        return results, None

    # Amortized timing: the axon/PJRT dispatch round-trip is ~70-80 ms and
    # dominates a single-call wall measurement, but dispatch pipelines, so
    # chained executions expose the true per-execution device time as the
    # marginal cost. Chain by donating the previous call's output buffers
    # (the kernel fully overwrites every output) so device-side execution
    # is strictly serialized.
    def run_chain(k):
        zs = fresh_zeros()
        t0 = time.perf_counter()
        o = tuple(zs)
        for _ in range(k):
            o = sharded(*dev_in, *o)
        jax.block_until_ready(o)
        return time.perf_counter() - t0

    K = 32
    w1 = min(run_chain(1) for _ in range(max(reps, 2)))
    wk = min(run_chain(K) for _ in range(max(reps, 2)))
    marginal = (wk - w1) / (K - 1)
    best = max(marginal, 1e-6)
    return results, int(best * 1e9)


def kernel(**inputs):
    import sys
    if "/opt/trn_rl_repo" not in sys.path:
        sys.path.insert(0, "/opt/trn_rl_repo")
    import concourse.bacc as bacc

    x = np.asarray(inputs["x"], np.float32)
    edge_index = np.asarray(inputs["edge_index"])
    curv = np.asarray(inputs["curvature_embeddings"], np.float32)
    weights = {k: np.asarray(v) for k, v in inputs.items()
               if k not in ("x", "edge_index", "curvature_embeddings")}

    pp, in_maps = host_prep(x, edge_index, curv, weights, NCORES, W_PER_CORE)
    nc = build_program(pp, lambda: bacc.Bacc(
        "TRN2", target_bir_lowering=False, debug=False, num_devices=NCORES))
    results, best_ns = _run_spmd_timed(nc, in_maps, NCORES)
    kernel.last_exec_ns = best_ns
    out = np.concatenate([results[c]["out"] for c in range(NCORES)],
                         axis=0)[:x.shape[0]]
    return np.ascontiguousarray(out, dtype=np.float32)



# revision 63
# speedup vs baseline: 1.2479x; 1.0145x over previous
"""Curvphormer GNN layer as a Bass/Tile SPMD kernel for TRN2 (V6).

Design (per core c of NCORES, owning 49 windows x 128 nodes):
 - Edges sharded by src range. Pass 1 groups edges by 256-node tgt
   wide-windows (ww), sorted by src within a ww so pass-2 runs are
   contiguous. Pass 2 groups edges by own src-window as runs of R=4
   consecutive pass-1 slots (one indirect DMA per run-group reads ex).
 - Phase A: batched LN stats; per-window xn^T via PE transpose; q/k/v as
   bf16 matmuls to DRAM. AllGather(k) and AllGather(v) (both with Shared
   pair-HBM outputs); the k "hi" table is an offset view k_full[NCUT:]
   so int16 gather indices stay in range (no copy).
 - Pass 1 (super-blocks of 8 wws): q/k rows via batched NON-transposed
   dma_gather (slot-major, contiguous 256B writes — much faster on HW
   than transposed gathers); prod = q*k on DVE; per-head dot via strided
   DVE tensor_reduce; score = qk + PSUM(curv@4Wc + 4bc) added on DVE;
   batched exp(0.25*s). Denominator via half-one-hot rows (i128z, 256B)
   gathered once per super-block + lo/hi-masked ex matmuls into PSUM.
 - AllReduce(den, fp32, Shared out); full gathered v normalized LOCALLY
   on DVE (no second AllGather exposed between pass 1 and pass 2).
 - Pass 2 (32-tile blocks): vn gathered in node-pairs (idx=tgt//2, 512B
   elems) with parity folded into ex masks; ex via indirect run-gathers;
   aggregation via iota/is_equal one-hot matmuls into per-window PSUM;
   out = x + agg@Wo + FFN(LN2(x1)) with transposed-chunk FFN.
"""

import sys
if "/opt/trn_rl_repo" not in sys.path:
    sys.path.insert(0, "/opt/trn_rl_repo")

import numpy as np

import concourse.bass as bass
import concourse.mybir as mybir
from concourse.masks import make_identity

F32 = mybir.dt.float32
BF16 = mybir.dt.bfloat16
I32 = mybir.dt.int32
I16 = mybir.dt.int16

D = 128
H = 8
HD = 16
LN_EPS = 1e-5
R2 = 4          # pass-2 run length (ex rows per indirect read)
NCUT = 17408    # k_hi table base (68 wide-windows * 256)


def _bf(a):
    import ml_dtypes
    return np.asarray(a, np.float32).astype(ml_dtypes.bfloat16)


def _wrap16(flat, ncols):
    """int16 idx table [128, ncols]: slot s -> [s%16, s//16], replicated."""
    tab = np.zeros((128, ncols), np.int16)
    tab[:16] = np.asarray(flat, np.int16).reshape(-1, 16).T
    for r in range(16, 128, 16):
        tab[r:r + 16] = tab[:16]
    return tab


class P:
    def __init__(self, ncores, W, T1W, NG2W):
        self.ncores = ncores
        self.W = W
        self.T1W = T1W
        self.NWW = ncores * W // 2
        self.T1 = self.NWW * T1W
        self.NG2W = NG2W
        self.T2W = NG2W * R2
        self.T2 = W * self.T2W
        self.NG2 = W * NG2W
        self.nodes_pc = W * 128
        self.npad = ncores * W * 128


# --------------------------------------------------------------------------
# Host-side preprocessing
# --------------------------------------------------------------------------

def host_prep(x, edge_index, curv, weights, ncores, W):
    N = x.shape[0]
    E = edge_index.shape[1]
    nodes_pc = W * 128
    npad = ncores * nodes_pc
    assert npad >= N

    src = np.asarray(edge_index[0], dtype=np.int64)
    tgt = np.asarray(edge_index[1], dtype=np.int64)
    x_pad = np.zeros((npad, D), dtype=np.float32)
    x_pad[:N] = x

    core_of = (src // 128) // W
    order_by_core = np.argsort(core_of, kind="stable")
    counts = np.bincount(core_of, minlength=ncores)
    splits = np.split(order_by_core, np.cumsum(counts)[:-1])

    NWW = ncores * W // 2

    # ---- pass-1 slot assignment (per core), sorted by (ww, src) ----
    T1W = 0
    p1_orders = []
    for c in range(ncores):
        e_c = splits[c]
        ww_of = tgt[e_c] // 256
        order = np.lexsort((src[e_c], ww_of))
        e_sorted = e_c[order]
        p1_orders.append(e_sorted)
        cnt = np.bincount(tgt[e_sorted] // 256, minlength=NWW)
        T1W = max(T1W, int(np.ceil(cnt.max() / 128)))
    T1 = NWW * T1W
    S1 = T1 * 128

    # per-core pass-1 tables
    core_p1 = []
    for c in range(ncores):
        e_sorted = p1_orders[c]
        ww_sorted = tgt[e_sorted] // 256
        cnt = np.bincount(ww_sorted, minlength=NWW)
        starts = NWW and np.concatenate([[0], np.cumsum(cnt)[:-1]])
        slots = np.zeros(len(e_sorted), np.int64)
        for ww in range(NWW):
            k = cnt[ww]
            if k:
                sl = ww * T1W * 128 + np.arange(k)
                slots[starts[ww]:starts[ww] + k] = sl
        real1 = np.zeros(S1, bool)
        real1[slots] = True
        tgt1 = np.zeros(S1, np.int64)
        tgt1[slots] = tgt[e_sorted]
        src1 = np.zeros(S1, np.int64)
        src1[slots] = src[e_sorted]
        core_p1.append((e_sorted, slots, real1, tgt1, src1))

    # ---- pass-2 runs (per core, per window) ----
    NG2W = 0
    core_runs = []
    for c in range(ncores):
        e_sorted, slots, real1, tgt1, src1 = core_p1[c]
        w_loc = (src[e_sorted] // 128) - c * W
        runs_per_w = []
        for w in range(W):
            s_list = np.sort(slots[w_loc == w])
            if len(s_list) == 0:
                runs_per_w.append([])
                continue
            d = np.diff(s_list)
            segstart = np.concatenate([[0], np.flatnonzero(d != 1) + 1])
            seglen = np.diff(np.concatenate([segstart, [len(s_list)]]))
            runs = []
            for ss, ln in zip(segstart, seglen):
                for off in range(0, ln, R2):
                    runs.append((int(s_list[ss + off]),
                                 int(min(R2, ln - off))))
            runs_per_w.append(runs)
            NG2W = max(NG2W, (len(runs) + 127) // 128)
        core_runs.append(runs_per_w)

    pp = P(ncores, W, T1W, NG2W)
    T2, T2W, NG2 = pp.T2, pp.T2W, pp.NG2
    S2 = T2 * 128

    # ---- weights (common) ----
    g1, be1 = weights["g1"], weights["be1"]
    g2, be2 = weights["g2"], weights["be2"]

    def foldA(Wm, b):
        return (_bf(g1[:, None] * Wm),
                _bf((be1 @ Wm + b))[None, :])

    wqp, bqp = foldA(weights["Wq"], weights["bq"])
    wkp, bkp = foldA(weights["Wk"], weights["bk"])
    wvp, bvp = foldA(weights["Wv"], weights["bv"])
    w1g = _bf(g2[:, None] * weights["W1"])
    b12 = (be2 @ weights["W1"] + weights["b1"]).astype(np.float32)
    b12cols = np.ascontiguousarray(b12.reshape(4, 128).T)  # [128, 4]
    w2ch = _bf(np.ascontiguousarray(
        weights["W2"].astype(np.float32).reshape(4, 128, D)
        .transpose(1, 0, 2).reshape(128, 4 * D)))

    headmask = np.zeros((128, H), np.float32)
    for f in range(128):
        headmask[f, f // 16] = 1.0
    i128z = np.zeros((129, 128), np.float32)
    i128z[:128, :128] = np.eye(128)
    iota128 = np.tile(np.arange(128, dtype=np.float32)[None, :], (128, 1))

    common = {
        "wqp": wqp, "wkp": wkp, "wvp": wvp,
        "bqp": bqp, "bkp": bkp, "bvp": bvp,
        "wc4": _bf(4.0 * weights["Wc"]),
        "bc4": _bf(4.0 * weights["bc"])[None, :],
        "wo_b": _bf(weights["Wo"]), "bo_b": _bf(weights["bo"])[None, :],
        "w1g": w1g, "b12cols": b12cols.astype(np.float32),
        "w2ch": w2ch, "b2_b": _bf(weights["b2"])[None, :],
        "ones_b": np.ones((1, D), np.float32).astype(np.float32),
        "headmask": _bf(headmask),
        "i128z": _bf(i128z),
        "iota128_b": _bf(iota128),
    }
    common["ones_b"] = _bf(np.ones((1, D), np.float32))

    in_maps = []
    for c in range(ncores):
        e_sorted, slots, real1, tgt1, src1 = core_p1[c]

        qi = np.where(real1, src1 - c * nodes_pc, 0)
        klo = np.where(tgt1 < NCUT, tgt1, tgt1 - NCUT)
        ki = np.where(real1, klo, 0)
        ohi = np.where(real1, tgt1 % 128, 128)
        is_lo = real1 & ((tgt1 % 256) < 128)
        is_hi = real1 & ((tgt1 % 256) >= 128)
        lobm = np.ascontiguousarray(
            is_lo.astype(np.float32).reshape(T1, 128).T)
        hibm = np.ascontiguousarray(
            is_hi.astype(np.float32).reshape(T1, 128).T)

        curv1 = np.zeros((S1, D), np.float32)
        curv1[slots] = curv[e_sorted]
        c1t = curv1.reshape(T1, 128, D).transpose(0, 2, 1)  # [T1, D, 128]
        curv1t = _bf(np.ascontiguousarray(
            c1t.reshape(T1 // 2, 2, D, 128).transpose(0, 2, 1, 3))
            .reshape((T1 // 2) * D, 256))

        # ---- pass 2 tables ----
        runs_per_w = core_runs[c]
        inv_slot1 = np.full(S1, -1, np.int64)
        inv_slot1[slots] = e_sorted
        vni = np.zeros(S2, np.int64)
        par = np.zeros(S2, np.float32)
        sl2 = np.full(S2, -1.0, np.float32)
        valid2 = np.zeros(S2, bool)
        exoff = np.zeros((128, NG2), np.int32)
        for w in range(W):
            runs = runs_per_w[w]
            for ri, (r0, ln) in enumerate(runs):
                g, p = ri // 128, ri % 128
                exoff[p, w * NG2W + g] = r0
                base_tile = (w * NG2W + g) * R2
                for t_ in range(ln):
                    e = inv_slot1[r0 + t_]
                    u = (base_tile + t_) * 128 + p
                    vni[u] = tgt[e] // 2
                    par[u] = float(tgt[e] % 2)
                    sl2[u] = float(src[e] - (c * W + w) * 128)
                    valid2[u] = True

        x_own = np.ascontiguousarray(x_pad[c * nodes_pc:(c + 1) * nodes_pc])

        m = dict(common)
        m.update({
            "x_own": x_own,
            "x_bf": _bf(x_own),
            "curv1t": curv1t,
            "lobm": _bf(lobm),
            "hibm": _bf(hibm),
            "qi16": _wrap16(qi, T1 * 8),
            "ki16": _wrap16(ki, T1 * 8),
            "ohi16": _wrap16(ohi, T1 * 8),
            "vni16": _wrap16(vni, T2 * 8),
            "exoff": exoff,
            "srcl2": _bf(np.ascontiguousarray(
                sl2.reshape(T2, 128).T)),
            "blm": _bf(np.ascontiguousarray(
                (valid2 & (par == 0)).astype(np.float32).reshape(T2, 128).T)),
            "bhm": _bf(np.ascontiguousarray(
                (valid2 & (par == 1)).astype(np.float32).reshape(T2, 128).T)),
        })
        in_maps.append(m)

    return pp, in_maps


# --------------------------------------------------------------------------
# Device program
# --------------------------------------------------------------------------

def declare_io(nc, pp):
    t = {}

    def din(name, shape, dt=F32):
        t[name] = nc.dram_tensor(name, list(shape), dt, kind="ExternalInput").ap()

    W, T1, T2 = pp.W, pp.T1, pp.T2
    din("x_own", (pp.nodes_pc, D))
    din("x_bf", (pp.nodes_pc, D), BF16)
    din("curv1t", ((T1 // 2) * 128, 2 * D), BF16)
    din("lobm", (128, T1), BF16)
    din("hibm", (128, T1), BF16)
    din("qi16", (128, T1 * 8), I16)
    din("ki16", (128, T1 * 8), I16)
    din("ohi16", (128, T1 * 8), I16)
    din("vni16", (128, T2 * 8), I16)
    din("exoff", (128, pp.NG2), I32)
    din("srcl2", (128, T2), BF16)
    din("blm", (128, T2), BF16)
    din("bhm", (128, T2), BF16)
    for n, shp, dt in [
            ("wqp", (D, D), BF16), ("wkp", (D, D), BF16), ("wvp", (D, D), BF16),
            ("bqp", (1, D), BF16), ("bkp", (1, D), BF16), ("bvp", (1, D), BF16),
            ("wc4", (D, H), BF16), ("bc4", (1, H), BF16),
            ("wo_b", (D, D), BF16), ("bo_b", (1, D), BF16),
            ("w1g", (D, 4 * D), BF16), ("b12cols", (128, 4), F32),
            ("w2ch", (D, 4 * D), BF16), ("b2_b", (1, D), BF16),
            ("ones_b", (1, D), BF16), ("headmask", (D, H), BF16),
            ("i128z", (129, 128), BF16), ("iota128_b", (128, 128), BF16)]:
        din(n, shp, dt)
    t["out"] = nc.dram_tensor("out", [pp.nodes_pc, D], F32,
                              kind="ExternalOutput").ap()
    return t


def build(tc, t, pp):
    nc = tc.nc
    W, T1W, T1, T2W, T2, NWW = (pp.W, pp.T1W, pp.T1, pp.T2W, pp.T2,
                                pp.NWW)
    NW = pp.ncores * W
    rg = [list(range(pp.ncores))]
    from contextlib import ExitStack
    ctx = ExitStack()

    # internal DRAM
    q_own_d, _ = tc.tile([pp.nodes_pc, D], BF16, space="DRAM", name="q_own_d")
    k_own_d, _ = tc.tile([pp.nodes_pc, D], BF16, space="DRAM", name="k_own_d")
    v_own_d, _ = tc.tile([pp.nodes_pc, D], BF16, space="DRAM", name="v_own_d")
    shpool = ctx.enter_context(tc.tile_pool(name="shdram", space="DRAM",
                                            bufs=1))
    k_full = shpool.tile([pp.npad, D], BF16, name="k_full",
                         addr_space="Shared")
    v_full = shpool.tile([pp.npad, D], BF16, name="v_full",
                         addr_space="Shared")
    den_d, _ = tc.tile([NW * 128, H], F32, space="DRAM", name="den_d")
    den_all = shpool.tile([NW * 128, H], F32, name="den_all",
                          addr_space="Shared")
    vn_full, _ = tc.tile([pp.npad, D], BF16, space="DRAM", name="vn_full")
    ex_d2, _ = tc.tile([T1 * 128 + 8, H], BF16, space="DRAM", name="ex_d2")

    const = ctx.enter_context(tc.tile_pool(name="const", bufs=1))

    def load_const(name):
        ap = t[name]
        tl = const.tile(list(ap.shape), ap.dtype, name=f"c_{name}")
        nc.sync.dma_start(tl[:], ap[:])
        return tl

    wqp_s = load_const("wqp"); wkp_s = load_const("wkp"); wvp_s = load_const("wvp")
    bqp_s = load_const("bqp"); bkp_s = load_const("bkp"); bvp_s = load_const("bvp")
    wc4_s = load_const("wc4"); bc4_s = load_const("bc4")
    wo_s = load_const("wo_b"); bo_s = load_const("bo_b")
    w1g_s = load_const("w1g"); b12c_s = load_const("b12cols")
    w2_s = load_const("w2ch"); b2_s = load_const("b2_b")
    ones_s = load_const("ones_b"); hmask_s = load_const("headmask")
    iota128_s = load_const("iota128_b")

    ident = const.tile([128, 128], F32, name="ident")
    make_identity(nc, ident[:])
    ident_b = const.tile([128, 128], BF16, name="ident_b")
    nc.vector.tensor_copy(out=ident_b[:], in_=ident[:])
    eps_col = const.tile([128, 1], F32, name="eps_col")
    nc.vector.memset(eps_col[:], LN_EPS)
    zrow = const.tile([8, H], BF16, name="zrow")
    nc.vector.memset(zrow[:], 0.0)

    # residents
    den_tab = const.tile([128, NWW * 2 * H], F32, name="den_tab")
    x1_res = const.tile([128, W * 128], F32, name="x1_res")

    # ---------------- Phase A ----------------
    with tc.tile_pool(name="pA", bufs=1) as pA, \
         tc.tile_pool(name="pAw", bufs=2) as pAw, \
         tc.tile_pool(name="pAp", bufs=2, space="PSUM") as pAp:
        xb = pA.tile([128, W * 128], BF16, tag="xb")
        nc.sync.dma_start(
            xb[:].rearrange("p (w f) -> p w f", w=W),
            t["x_bf"][:].rearrange("(w p) f -> p w f", p=128))
        xv = xb[:].rearrange("p (w f) -> p w f", w=W)
        s1 = pA.tile([128, W], F32, tag="s1")
        nc.vector.tensor_reduce(out=s1[:], in_=xv, axis=mybir.AxisListType.X,
                                op=mybir.AluOpType.add)
        sq = pA.tile([128, W * 128], BF16, tag="sq")
        nc.scalar.activation(out=sq[:], in_=xb[:],
                             func=mybir.ActivationFunctionType.Square)
        s2 = pA.tile([128, W], F32, tag="s2")
        nc.vector.tensor_reduce(out=s2[:],
                                in_=sq[:].rearrange("p (w f) -> p w f", w=W),
                                axis=mybir.AxisListType.X,
                                op=mybir.AluOpType.add)
        mcol = pA.tile([128, W], F32, tag="mcol")
        nc.vector.tensor_scalar_mul(mcol[:], s1[:], 1.0 / 128.0)
        m2c = pA.tile([128, W], F32, tag="m2c")
        nc.vector.tensor_tensor(out=m2c[:], in0=mcol[:], in1=mcol[:],
                                op=mybir.AluOpType.mult)
        var = pA.tile([128, W], F32, tag="var")
        nc.vector.scalar_tensor_tensor(out=var[:], in0=s2[:],
                                       scalar=1.0 / 128.0, in1=m2c[:],
                                       op0=mybir.AluOpType.mult,
                                       op1=mybir.AluOpType.subtract)
        stdc = pA.tile([128, W], F32, tag="stdc")
        nc.scalar.activation(out=stdc[:], in_=var[:],
                             func=mybir.ActivationFunctionType.Sqrt,
                             bias=eps_col[:])
        rstd = pA.tile([128, W], F32, tag="rstd")
        nc.vector.reciprocal(out=rstd[:], in_=stdc[:])
        negm = pA.tile([128, W], F32, tag="negm")
        nc.vector.tensor_scalar_mul(negm[:], mcol[:], -1.0)
        xnt = pA.tile([128, W * 128], BF16, tag="xnt")
        nc.vector.tensor_tensor(
            out=xnt[:].rearrange("p (w f) -> p w f", w=W), in0=xv,
            in1=negm[:].rearrange("p w -> p w ()").broadcast_to([128, W, 128]),
            op=mybir.AluOpType.add)
        xn = pA.tile([128, W * 128], BF16, tag="xn")
        nc.vector.tensor_tensor(
            out=xn[:].rearrange("p (w f) -> p w f", w=W),
            in0=xnt[:].rearrange("p (w f) -> p w f", w=W),
            in1=rstd[:].rearrange("p w -> p w ()").broadcast_to([128, W, 128]),
            op=mybir.AluOpType.mult)

        for w in range(W):
            xnT_ps = pAp.tile([128, 128], BF16, tag="xnT_ps")
            nc.tensor.transpose(out=xnT_ps[:], in_=xn[:, w * 128:(w + 1) * 128],
                                identity=ident_b[:])
            xnT = pAw.tile([128, 128], BF16, tag="xnT")
            nc.vector.tensor_copy(out=xnT[:], in_=xnT_ps[:])
            for nm, wmat, brow, dst in (
                    ("q", wqp_s, bqp_s, q_own_d), ("k", wkp_s, bkp_s, k_own_d),
                    ("v", wvp_s, bvp_s, v_own_d)):
                ps = pAp.tile([128, 128], F32, tag="ps")
                nc.tensor.matmul(out=ps[:], lhsT=xnT[:], rhs=wmat[:],
                                 start=True, stop=False)
                nc.tensor.matmul(out=ps[:], lhsT=ones_s[:], rhs=brow[:],
                                 start=False, stop=True)
                ot = pAw.tile([128, 128], BF16, tag=f"o_{nm}")
                nc.scalar.activation(out=ot[:], in_=ps[:],
                                     func=mybir.ActivationFunctionType.Copy)
                nc.sync.dma_start(dst[w * 128:(w + 1) * 128, :], ot[:])

    nc.gpsimd.collective_compute(
        "AllGather", mybir.AluOpType.bypass, replica_groups=rg,
        ins=[k_own_d.opt()], outs=[k_full.opt()])
    nc.gpsimd.collective_compute(
        "AllGather", mybir.AluOpType.bypass, replica_groups=rg,
        ins=[v_own_d.opt()], outs=[v_full.opt()])

    # ---------------- Pass 1 ----------------
    NBLK = NWW // 2
    nt1 = 2 * T1W
    BB = 4                        # blocks per super-block
    NSB = (NBLK + BB - 1) // BB
    WWCUT = NCUT // 256           # first hi wide-window
    k_hi_v = k_full[NCUT:pp.npad, :]
    with tc.tile_pool(name="p1", bufs=2) as p1, \
         tc.tile_pool(name="p1p", bufs=2, space="PSUM") as p1p, \
         tc.tile_pool(name="p1d", bufs=2, space="PSUM") as p1d:
        nc.sync.dma_start(ex_d2[T1 * 128:T1 * 128 + 8, :], zrow[:])
        for sbi in range(NSB):
            b0 = sbi * BB
            nb = min(BB, NBLK - b0)
            nt = nb * nt1
            t0 = b0 * nt1
            s0 = t0 * 128
            ni = nt * 128
            cvb = p1.tile([128, BB * nt1 * 128], BF16, tag="cvb")
            nc.sync.dma_start(
                cvb[:, :ni].rearrange("p (b e) -> p b e", e=256),
                t["curv1t"][(t0 // 2) * 128:((t0 + nt) // 2) * 128, :]
                .rearrange("(b p) e -> p b e", p=128))
            qi_s = p1.tile([128, BB * nt1 * 8], I16, tag="qi_s")
            nc.sync.dma_start(qi_s[:, :nt * 8],
                              t["qi16"][:, s0 // 16:(s0 + ni) // 16])
            ki_s = p1.tile([128, BB * nt1 * 8], I16, tag="ki_s")
            nc.sync.dma_start(ki_s[:, :nt * 8],
                              t["ki16"][:, s0 // 16:(s0 + ni) // 16])
            ohi_s = p1.tile([128, BB * nt1 * 8], I16, tag="ohi_s")
            nc.sync.dma_start(ohi_s[:, :nt * 8],
                              t["ohi16"][:, s0 // 16:(s0 + ni) // 16])
            lob_s = p1.tile([128, BB * nt1], BF16, tag="lob_s")
            nc.sync.dma_start(lob_s[:, :nt], t["lobm"][:, t0:t0 + nt])
            hib_s = p1.tile([128, BB * nt1], BF16, tag="hib_s")
            nc.sync.dma_start(hib_s[:, :nt], t["hibm"][:, t0:t0 + nt])
            qT = p1.tile([128, BB * nt1 * 128], BF16, tag="qT")
            nc.gpsimd.dma_gather(
                out_ap=qT[:, :ni].rearrange("p (i e) -> p i e", i=nt),
                in_ap=q_own_d[:], idxs_ap=qi_s[:, :ni // 16],
                num_idxs=ni, num_idxs_reg=ni, elem_size=128,
                single_packet=False)
            kT = p1.tile([128, BB * nt1 * 128], BF16, tag="kT")
            ww_lo, ww_hi = 2 * b0, 2 * (b0 + nb)
            segs = []
            if ww_lo < WWCUT:
                segs.append((ww_lo, min(ww_hi, WWCUT), k_full[:]))
            if ww_hi > WWCUT:
                segs.append((max(ww_lo, WWCUT), ww_hi, k_hi_v))
            for (wa, wb, ktab_ap) in segs:
                ta = (wa - ww_lo) * T1W
                tb = (wb - ww_lo) * T1W
                na = (tb - ta) * 128
                nc.gpsimd.dma_gather(
                    out_ap=kT[:, ta * 128:tb * 128]
                    .rearrange("p (i e) -> p i e", i=tb - ta),
                    in_ap=ktab_ap,
                    idxs_ap=ki_s[:, ta * 8:tb * 8],
                    num_idxs=na, num_idxs_reg=na, elem_size=128,
                    single_packet=False)
            ohb = p1.tile([128, BB * nt1 * 128], BF16, tag="ohb")
            nc.gpsimd.dma_gather(
                out_ap=ohb[:, :ni].rearrange("p (i e) -> p i e", i=nt),
                in_ap=t["i128z"][:], idxs_ap=ohi_s[:, :ni // 16],
                num_idxs=ni, num_idxs_reg=ni, elem_size=128,
                single_packet=False)
            prodT = p1.tile([128, BB * nt1 * 128], BF16, tag="prodT")
            nc.vector.tensor_tensor(out=prodT[:, :ni], in0=qT[:, :ni],
                                    in1=kT[:, :ni], op=mybir.AluOpType.mult)
            qkred = p1.tile([128, BB * nt1 * 8], F32, tag="qkred")
            nc.vector.tensor_reduce(
                out=qkred[:, :nt * 8].rearrange("p (b h) -> p b h", h=H),
                in_=prodT[:, :ni].rearrange("p (b h x) -> p b h x",
                                            b=nt, h=H),
                axis=mybir.AxisListType.X, op=mybir.AluOpType.add)
            exb = p1.tile([128, BB * nt1 * 8], BF16, tag="exb")
            for bl in range(nb):
                sc_ps = p1p.tile([128, nt1 * 8], F32, tag="sc_ps")
                for j0 in range(nt1):
                    j = bl * nt1 + j0
                    scj = sc_ps[:, j0 * 8:(j0 + 1) * 8]
                    nc.tensor.matmul(out=scj,
                                     lhsT=cvb[:, j * 128:(j + 1) * 128],
                                     rhs=wc4_s[:], start=True, stop=False)
                    nc.tensor.matmul(out=scj, lhsT=ones_s[:], rhs=bc4_s[:],
                                     start=False, stop=True)
                s_sb = p1.tile([128, nt1 * 8], F32, tag="s_sb")
                nc.vector.tensor_tensor(
                    out=s_sb[:],
                    in0=qkred[:, bl * nt1 * 8:(bl + 1) * nt1 * 8],
                    in1=sc_ps[:], op=mybir.AluOpType.add)
                nc.scalar.activation(
                    out=exb[:, bl * nt1 * 8:(bl + 1) * nt1 * 8],
                    in_=s_sb[:], func=mybir.ActivationFunctionType.Exp,
                    scale=0.25)
            exl = p1.tile([128, BB * nt1 * 8], BF16, tag="exl")
            exh = p1.tile([128, BB * nt1 * 8], BF16, tag="exh")
            nc.vector.tensor_tensor(
                out=exl[:, :nt * 8].rearrange("p (b h) -> p b h", h=H),
                in0=exb[:, :nt * 8].rearrange("p (b h) -> p b h", h=H),
                in1=lob_s[:, :nt].rearrange("p b -> p b ()")
                .broadcast_to([128, nt, H]),
                op=mybir.AluOpType.mult)
            nc.vector.tensor_tensor(
                out=exh[:, :nt * 8].rearrange("p (b h) -> p b h", h=H),
                in0=exb[:, :nt * 8].rearrange("p (b h) -> p b h", h=H),
                in1=hib_s[:, :nt].rearrange("p b -> p b ()")
                .broadcast_to([128, nt, H]),
                op=mybir.AluOpType.mult)
            ohv = ohb[:, :ni].rearrange("p (i e) -> p i e", i=nt)
            for bl in range(nb):
                for i in range(2):
                    ww = 2 * (b0 + bl) + i
                    psd_lo = p1d.tile([128, H], F32, tag="psd_lo",
                                      name="psd_lo")
                    psd_hi = p1d.tile([128, H], F32, tag="psd_hi",
                                      name="psd_hi")
                    for tt in range(T1W):
                        jj = bl * nt1 + i * T1W + tt
                        nc.tensor.matmul(out=psd_lo[:], lhsT=ohv[:, jj, :],
                                         rhs=exl[:, jj * 8:(jj + 1) * 8],
                                         start=(tt == 0),
                                         stop=(tt == T1W - 1))
                        nc.tensor.matmul(out=psd_hi[:], lhsT=ohv[:, jj, :],
                                         rhs=exh[:, jj * 8:(jj + 1) * 8],
                                         start=(tt == 0),
                                         stop=(tt == T1W - 1))
                    nc.vector.tensor_copy(
                        out=den_tab[:, ww * 2 * H:ww * 2 * H + H],
                        in_=psd_lo[:])
                    nc.vector.tensor_copy(
                        out=den_tab[:, ww * 2 * H + H:(ww + 1) * 2 * H],
                        in_=psd_hi[:])
            nc.sync.dma_start(
                ex_d2[s0:s0 + ni, :].rearrange("(b p) h -> p b h", p=128),
                exb[:, :nt * 8].rearrange("p (b h) -> p b h", h=H))

        nc.sync.dma_start(
            den_d[:].rearrange("(w p) h -> p w h", p=128),
            den_tab[:].rearrange("p (w h) -> p w h", h=H))

    nc.gpsimd.collective_compute(
        "AllReduce", mybir.AluOpType.add, replica_groups=rg,
        ins=[den_d.opt()], outs=[den_all.opt()])

    # ---------------- Phase C: normalize full gathered v locally ---------
    with tc.tile_pool(name="pC", bufs=2) as pC:
        CHV = 8192
        nchv = (pp.npad + CHV - 1) // CHV
        for ch in range(nchv):
            r0 = ch * CHV
            nr = min(CHV, pp.npad - r0)
            na = nr // 128
            vb = pC.tile([128, (CHV // 128) * 128], BF16, tag="vb")
            nc.sync.dma_start(
                vb[:, :na * 128].rearrange("p (a f) -> p a f", a=na),
                v_full[r0:r0 + nr, :].rearrange("(a p) f -> p a f", p=128))
            db = pC.tile([128, (CHV // 128) * H], F32, tag="db")
            nc.sync.dma_start(
                db[:, :na * H].rearrange("p (a h) -> p a h", a=na),
                den_all[r0:r0 + nr, :].rearrange("(a p) h -> p a h", p=128))
            nc.vector.tensor_scalar_max(db[:, :na * H], db[:, :na * H],
                                        1e-30)
            rec = pC.tile([128, (CHV // 128) * H], F32, tag="rec")
            nc.vector.reciprocal(out=rec[:, :na * H], in_=db[:, :na * H])
            vnb = pC.tile([128, (CHV // 128) * 128], BF16, tag="vnb")
            nc.vector.tensor_tensor(
                out=vnb[:, :na * 128]
                .rearrange("p (a h x) -> p a h x", a=na, h=H),
                in0=vb[:, :na * 128]
                .rearrange("p (a h x) -> p a h x", a=na, h=H),
                in1=rec[:, :na * H].rearrange("p (a h) -> p a h ()", a=na)
                .broadcast_to([128, na, H, HD]),
                op=mybir.AluOpType.mult)
            nc.sync.dma_start(
                vn_full[r0:r0 + nr, :].rearrange("(a p) f -> p a f", p=128),
                vnb[:, :na * 128].rearrange("p (a f) -> p a f", a=na))

    # ---------------- Pass 2 ----------------
    B2 = 32
    NB2 = (T2 + B2 - 1) // B2
    vn_pair = vn_full[:].rearrange("(a b) f -> a (b f)", b=2)
    with tc.tile_pool(name="p2", bufs=2) as p2, \
         tc.tile_pool(name="p2c", bufs=1) as p2c, \
         tc.tile_pool(name="p2p", bufs=2, space="PSUM") as p2p, \
         tc.tile_pool(name="p2a", bufs=2, space="PSUM") as p2a, \
         tc.tile_pool(name="pD", bufs=2) as pD:
        vni_s = p2c.tile([128, T2 * 8], I16, name="vni_s")
        nc.sync.dma_start(vni_s[:], t["vni16"][:])
        exoff_s = p2c.tile([128, pp.NG2], I32, name="exoff_s")
        nc.sync.dma_start(exoff_s[:], t["exoff"][:])
        srcl2_s = p2c.tile([128, T2], BF16, name="srcl2_s")
        nc.sync.dma_start(srcl2_s[:], t["srcl2"][:])
        blm_s = p2c.tile([128, T2], BF16, name="blm_s")
        nc.sync.dma_start(blm_s[:], t["blm"][:])
        bhm_s = p2c.tile([128, T2], BF16, name="bhm_s")
        nc.sync.dma_start(bhm_s[:], t["bhm"][:])

        aggT_cur = [None]
        for bi in range(NB2):
            t0 = bi * B2
            nt = min(B2, T2 - t0)
            s0 = t0 * 128
            ni = nt * 128
            vgbp = p2.tile([128, B2 * 256], BF16, tag="vgbp")
            nc.gpsimd.dma_gather(
                out_ap=vgbp[:, :nt * 256].rearrange("p (i e) -> p i e", i=nt),
                in_ap=vn_pair, idxs_ap=vni_s[:, s0 // 16:(s0 + ni) // 16],
                num_idxs=ni, num_idxs_reg=ni, elem_size=256,
                single_packet=False)
            egb = p2.tile([128, B2 * 8], BF16, tag="egb")
            ng = (nt + R2 - 1) // R2
            for gi in range(ng):
                g = t0 // R2 + gi
                nc.gpsimd.indirect_dma_start(
                    out=egb[:, gi * R2 * 8:(gi + 1) * R2 * 8],
                    out_offset=None,
                    in_=ex_d2[:],
                    in_offset=bass.IndirectOffsetOnAxis(
                        ap=exoff_s[:, g:g + 1], axis=0))
            exbl = p2.tile([128, B2 * 8], BF16, tag="exbl")
            nc.vector.tensor_tensor(
                out=exbl[:, :nt * 8].rearrange("p (b h) -> p b h", b=nt),
                in0=egb[:, :nt * 8].rearrange("p (b h) -> p b h", b=nt),
                in1=blm_s[:, t0:t0 + nt].rearrange("p b -> p b ()")
                .broadcast_to([128, nt, H]),
                op=mybir.AluOpType.mult)
            exbh = p2.tile([128, B2 * 8], BF16, tag="exbh")
            nc.vector.tensor_tensor(
                out=exbh[:, :nt * 8].rearrange("p (b h) -> p b h", b=nt),
                in0=egb[:, :nt * 8].rearrange("p (b h) -> p b h", b=nt),
                in1=bhm_s[:, t0:t0 + nt].rearrange("p b -> p b ()")
                .broadcast_to([128, nt, H]),
                op=mybir.AluOpType.mult)
            vv = vgbp[:].rearrange("p (i e) -> p i e", i=B2)
            msglo = p2.tile([128, B2 * 128], BF16, tag="msglo")
            nc.vector.tensor_tensor(
                out=msglo[:, :nt * 128]
                .rearrange("p (b h x) -> p b h x", b=nt, h=H),
                in0=vv[:, :nt, 0:128].rearrange("p b (h x) -> p b h x", h=H),
                in1=exbl[:, :nt * 8].rearrange("p (b h) -> p b h ()", b=nt)
                .broadcast_to([128, nt, H, HD]),
                op=mybir.AluOpType.mult)
            msghi = p2.tile([128, B2 * 128], BF16, tag="msghi")
            nc.vector.tensor_tensor(
                out=msghi[:, :nt * 128]
                .rearrange("p (b h x) -> p b h x", b=nt, h=H),
                in0=vv[:, :nt, 128:256].rearrange("p b (h x) -> p b h x", h=H),
                in1=exbh[:, :nt * 8].rearrange("p (b h) -> p b h ()", b=nt)
                .broadcast_to([128, nt, H, HD]),
                op=mybir.AluOpType.mult)
            oh2b = p2.tile([128, B2 * 128], BF16, tag="oh2b")
            nc.vector.tensor_tensor(
                out=oh2b[:, :nt * 128].rearrange("p (b e) -> p b e", b=nt),
                in0=srcl2_s[:, t0:t0 + nt].rearrange("p b -> p b ()")
                .broadcast_to([128, nt, 128]),
                in1=iota128_s[:].rearrange("p e -> p () e")
                .broadcast_to([128, nt, 128]),
                op=mybir.AluOpType.is_equal)
            for j in range(nt):
                tj = t0 + j
                w = tj // T2W
                tt = tj % T2W
                if tt == 0:
                    aggT_cur[0] = p2a.tile([128, 128], F32, tag="aggT",
                                           name="aggT")
                aggT = aggT_cur[0]
                nc.tensor.matmul(out=aggT[:],
                                 lhsT=msglo[:, j * 128:(j + 1) * 128],
                                 rhs=oh2b[:, j * 128:(j + 1) * 128],
                                 start=(tt == 0), stop=False)
                nc.tensor.matmul(out=aggT[:],
                                 lhsT=msghi[:, j * 128:(j + 1) * 128],
                                 rhs=oh2b[:, j * 128:(j + 1) * 128],
                                 start=False, stop=(tt == T2W - 1))
                if tt == T2W - 1:
                    aggT_sb = pD.tile([128, 128], BF16, tag="aggT_sb")
                    nc.vector.tensor_copy(out=aggT_sb[:], in_=aggT[:])
                    attn = p2p.tile([128, 128], F32, tag="attn")
                    nc.tensor.matmul(out=attn[:], lhsT=aggT_sb[:],
                                     rhs=wo_s[:], start=True, stop=False)
                    nc.tensor.matmul(out=attn[:], lhsT=ones_s[:],
                                     rhs=bo_s[:], start=False, stop=True)
                    xw2 = pD.tile([128, 128], F32, tag="xw2")
                    nc.sync.dma_start(xw2[:],
                                      t["x_own"][w * 128:(w + 1) * 128, :])
                    nc.vector.tensor_tensor(
                        out=x1_res[:, w * 128:(w + 1) * 128],
                        in0=xw2[:], in1=attn[:], op=mybir.AluOpType.add)

    # ---------------- Phase D ----------------
    with tc.tile_pool(name="pDm", bufs=1) as pDm, \
         tc.tile_pool(name="pDw", bufs=2) as pDw, \
         tc.tile_pool(name="pDp", bufs=2, space="PSUM") as pDp, \
         tc.tile_pool(name="pDh", bufs=2, space="PSUM") as pDh:
        x1v = x1_res[:].rearrange("p (w f) -> p w f", w=W)
        s1b = pDm.tile([128, W], F32, tag="s1b")
        nc.vector.tensor_reduce(out=s1b[:], in_=x1v, axis=mybir.AxisListType.X,
                                op=mybir.AluOpType.add)
        sqb = pDm.tile([128, W * 128], BF16, tag="sqb")
        nc.scalar.activation(out=sqb[:], in_=x1_res[:],
                             func=mybir.ActivationFunctionType.Square)
        s2b = pDm.tile([128, W], F32, tag="s2b")
        nc.vector.tensor_reduce(out=s2b[:],
                                in_=sqb[:].rearrange("p (w f) -> p w f", w=W),
                                axis=mybir.AxisListType.X,
                                op=mybir.AluOpType.add)
        mb = pDm.tile([128, W], F32, tag="mb")
        nc.vector.tensor_scalar_mul(mb[:], s1b[:], 1.0 / 128.0)
        m2b = pDm.tile([128, W], F32, tag="m2b")
        nc.vector.tensor_tensor(out=m2b[:], in0=mb[:], in1=mb[:],
                                op=mybir.AluOpType.mult)
        varb = pDm.tile([128, W], F32, tag="varb")
        nc.vector.scalar_tensor_tensor(out=varb[:], in0=s2b[:],
                                       scalar=1.0 / 128.0, in1=m2b[:],
                                       op0=mybir.AluOpType.mult,
                                       op1=mybir.AluOpType.subtract)
        stdb = pDm.tile([128, W], F32, tag="stdb")
        nc.scalar.activation(out=stdb[:], in_=varb[:],
                             func=mybir.ActivationFunctionType.Sqrt,
                             bias=eps_col[:])
        rstdb = pDm.tile([128, W], F32, tag="rstdb")
        nc.vector.reciprocal(out=rstdb[:], in_=stdb[:])
        negmb = pDm.tile([128, W], F32, tag="negmb")
        nc.vector.tensor_scalar_mul(negmb[:], mb[:], -1.0)
        x1t = pDm.tile([128, W * 128], BF16, tag="x1t")
        nc.vector.tensor_tensor(
            out=x1t[:].rearrange("p (w f) -> p w f", w=W), in0=x1v,
            in1=negmb[:].rearrange("p w -> p w ()").broadcast_to([128, W, 128]),
            op=mybir.AluOpType.add)
        x1n = pDm.tile([128, W * 128], BF16, tag="x1n")
        nc.vector.tensor_tensor(
            out=x1n[:].rearrange("p (w f) -> p w f", w=W),
            in0=x1t[:].rearrange("p (w f) -> p w f", w=W),
            in1=rstdb[:].rearrange("p w -> p w ()").broadcast_to([128, W, 128]),
            op=mybir.AluOpType.mult)

        for w in range(W):
            x1nT_ps = pDp.tile([128, 128], BF16, tag="x1nT_ps")
            nc.tensor.transpose(out=x1nT_ps[:],
                                in_=x1n[:, w * 128:(w + 1) * 128],
                                identity=ident_b[:])
            x1nT = pDw.tile([128, 128], BF16, tag="x1nT")
            nc.vector.tensor_copy(out=x1nT[:], in_=x1nT_ps[:])
            hsbT = pDw.tile([128, 4 * 128], BF16, tag="hsbT")
            for ch in range(4):
                hp = pDh.tile([128, 128], F32, tag="hp")
                nc.tensor.matmul(out=hp[:],
                                 lhsT=w1g_s[:, ch * 128:(ch + 1) * 128],
                                 rhs=x1nT[:], start=True, stop=True)
                nc.scalar.activation(out=hsbT[:, ch * 128:(ch + 1) * 128],
                                     in_=hp[:],
                                     func=mybir.ActivationFunctionType.Relu,
                                     bias=b12c_s[:, ch:ch + 1])
            ffn = pDp.tile([128, 128], F32, tag="ffn")
            for ch in range(4):
                nc.tensor.matmul(out=ffn[:],
                                 lhsT=hsbT[:, ch * 128:(ch + 1) * 128],
                                 rhs=w2_s[:, ch * 128:(ch + 1) * 128],
                                 start=(ch == 0), stop=False)
            nc.tensor.matmul(out=ffn[:], lhsT=ones_s[:], rhs=b2_s[:],
                             start=False, stop=True)
            outw = pDw.tile([128, 128], F32, tag="outw")
            nc.vector.tensor_tensor(out=outw[:],
                                    in0=x1_res[:, w * 128:(w + 1) * 128],
                                    in1=ffn[:], op=mybir.AluOpType.add)
            nc.sync.dma_start(t["out"][w * 128:(w + 1) * 128, :], outw[:])

    ctx.close()


def build_program(pp, nc_factory):
    import concourse.tile as tile
    nc = nc_factory()
    t = declare_io(nc, pp)
    with tile.TileContext(nc) as tc:
        build(tc, t, pp)
    nc.compile()
    return nc


# --------------------------------------------------------------------------
# Harness entry point
# --------------------------------------------------------------------------

NCORES = 8
W_PER_CORE = 49  # 8*49*128 = 50176 >= 50000 nodes


def _run_spmd_timed(nc, in_maps, n_cores, reps=4):
    """Execute the SPMD program via PJRT with device-staged inputs; returns
    (per-core results, estimated per-execution device ns)."""
    import time

    import jax
    from jax.experimental.shard_map import shard_map
    from jax.sharding import Mesh, NamedSharding, PartitionSpec

    from concourse.bass2jax import (_bass_exec_p, install_neuronx_cc_hook,
                                    partition_id_tensor)

    install_neuronx_cc_hook()
    partition_name = (nc.partition_id_tensor.name
                      if nc.partition_id_tensor else None)
    in_names, out_names, out_avals, zero_outs = [], [], [], []
    for alloc in nc.m.functions[0].allocations:
        if not isinstance(alloc, mybir.MemoryLocationSet):
            continue
        name = alloc.memorylocations[0].name
        if alloc.kind == "ExternalInput":
            if name != partition_name:
                in_names.append(name)
        elif alloc.kind == "ExternalOutput":
            shape = tuple(alloc.tensor_shape)
            dtype = mybir.dt.np(alloc.dtype)
            out_names.append(name)
            out_avals.append(jax.core.ShapedArray(shape, dtype))
            zero_outs.append(np.zeros(shape, dtype))
    n_params = len(in_names)
    n_outs = len(out_avals)
    in_names.extend(out_names)
    if partition_name is not None:
        in_names.append(partition_name)
    donate = tuple(range(n_params, n_params + n_outs))

    def _body(*args):
        operands = list(args)
        if partition_name is not None:
            operands.append(partition_id_tensor())
        outs = _bass_exec_p.bind(
            *operands, out_avals=tuple(out_avals), in_names=tuple(in_names),
            out_names=tuple(out_names), lowering_input_output_aliases=(),
            sim_require_finite=True, sim_require_nnan=True, nc=nc)
        return tuple(outs)

    devices = jax.devices()[:n_cores]
    mesh = Mesh(np.asarray(devices), ("core",))
    sharding = NamedSharding(mesh, PartitionSpec("core"))
    in_specs = (PartitionSpec("core"),) * (n_params + n_outs)
    out_specs = (PartitionSpec("core"),) * len(out_names)
    sharded = jax.jit(
        shard_map(_body, mesh=mesh, in_specs=in_specs, out_specs=out_specs,
                  check_rep=False),
        donate_argnums=donate, keep_unused=True)
    concat_in = [
        np.concatenate([np.asarray(in_maps[c][in_names[i]])
                        for c in range(n_cores)], axis=0)
        for i in range(n_params)]
    dev_in = [jax.device_put(a, sharding) for a in concat_in]

    def fresh_zeros():
        zs = [jax.device_put(
            np.zeros((n_cores * z.shape[0], *z.shape[1:]), z.dtype), sharding)
            for z in zero_outs]
        jax.block_until_ready(zs)
        return zs

    out_arrs = sharded(*dev_in, *fresh_zeros())
    jax.block_until_ready(out_arrs)
    results = [
        {name: np.asarray(out_arrs[i]).reshape(n_cores, *out_avals[i].shape)[c]
         for i, name in enumerate(out_names)}
        for c in range(n_cores)]
    if reps <= 0:
        return results, None

    # Amortized timing: the axon/PJRT dispatch round-trip is ~70-80 ms and
    # dominates a single-call wall measurement, but dispatch pipelines, so
    # chained executions expose the true per-execution device time as the
    # marginal cost. Chain by donating the previous call's output buffers
    # (the kernel fully overwrites every output) so device-side execution
    # is strictly serialized.
    def run_chain(k):
        zs = fresh_zeros()
        t0 = time.perf_counter()
        o = tuple(zs)
        for _ in range(k):
            o = sharded(*dev_in, *o)
        jax.block_until_ready(o)
        return time.perf_counter() - t0

    K = 32
    w1 = min(run_chain(1) for _ in range(max(reps, 2)))
    wk = min(run_chain(K) for _ in range(max(reps, 2)))
    marginal = (wk - w1) / (K - 1)
    best = max(marginal, 1e-6)
    return results, int(best * 1e9)


def kernel(**inputs):
    import sys
    if "/opt/trn_rl_repo" not in sys.path:
        sys.path.insert(0, "/opt/trn_rl_repo")
    import concourse.bacc as bacc

    x = np.asarray(inputs["x"], np.float32)
    edge_index = np.asarray(inputs["edge_index"])
    curv = np.asarray(inputs["curvature_embeddings"], np.float32)
    weights = {k: np.asarray(v) for k, v in inputs.items()
               if k not in ("x", "edge_index", "curvature_embeddings")}

    pp, in_maps = host_prep(x, edge_index, curv, weights, NCORES, W_PER_CORE)
    nc = build_program(pp, lambda: bacc.Bacc(
        "TRN2", target_bir_lowering=False, debug=False, num_devices=NCORES))
    results, best_ns = _run_spmd_timed(nc, in_maps, NCORES)
    kernel.last_exec_ns = best_ns
    out = np.concatenate([results[c]["out"] for c in range(NCORES)],
                         axis=0)[:x.shape[0]]
    return np.ascontiguousarray(out, dtype=np.float32)

